# revision 1
# baseline (speedup 1.0000x reference)
"""Tensor-parallel GQA multi-head attention for 8 Trainium2 NeuronCores.

Sharding: query heads (16) split 2-per-core; each core needs exactly one
KV head (GQA group); wq/wk/wv column-parallel, wo row-parallel; the
all-reduce after wo is done host-side (sum of 8 partial outputs).

Per-core layout strategy: activations kept transposed (feature dim on
partitions, tokens on the free axis) so every matmul contracts over the
partition dim with N=512 streams:
  QT/KT = W^T-chunks (lhsT) x xT (rhs)         [dh, tokens]
  S^T   = KT-chunk (lhsT) x QT (rhs)           [s, t]  (causal superblocks)
  P^T   = exp(S^T + causal mask)               (no max-subtraction: scores
                                                are bounded ~N(0, 1/9))
  l     = ones x P^T (column sums via PE)      [1, t]
  avT   = V-chunk (lhsT) x P^T (rhs)           [dh, t]; scaled by 1/l
  out   = avT-chunk (lhsT) x woT (rhs)         [t, d] partial, DMA'd out
"""

import numpy as np

B, T, D, H, KV = 2, 2048, 2048, 16, 4
DH = 128
NCORES = 8
HPC = H // NCORES          # 2 query heads per core
BT = B * T                 # 4096
ND = D // 128              # 16 contraction chunks
NSB = T // 512             # 4 causal superblocks per batch
NTC = BT // 128            # 32 output token chunks
ROPE_BASE = 10000.0
NEG = -1.0e4

_cache = {}


def _ensure_path():
    try:
        import concourse.bass  # noqa: F401
    except ImportError:
        import sys
        for p in ("/opt/trn_rl_repo", "/root/.axon_site/_ro/trn_rl_repo"):
            if p not in sys.path:
                sys.path.insert(0, p)
        import concourse.bass  # noqa: F401


def _split_multi_waits(nc, mybir, max_waits=1):
    """This container's walrus rejects >1 sync-wait on one instruction
    (seen on the Tile tail drain). Move extra waits onto preceding NoOps
    on the same engine; per-engine program order preserves semantics."""
    for bb in nc.main_func.blocks:
        new_insts = []
        for ins in bb.instructions:
            si = getattr(ins, "sync_info", None)
            if si is not None and si.on_wait and len(si.on_wait) > max_waits:
                waits = list(si.on_wait)
                extra, keep = waits[:-max_waits], waits[-max_waits:]
                for w in extra:
                    new_insts.append(
                        mybir.InstNoOp(
                            name=nc.get_next_instruction_name(),
                            sync_info=mybir.SyncInfo(on_wait=[w], on_update=[]),
                            bass_nofuse=True,
                            engine=ins.engine,
                            ins=[],
                            outs=[],
                        )
                    )
                si.on_wait = keep
            new_insts.append(ins)
        bb.instructions = new_insts


def _build(split_waits=True, use_f32r=True):
    _ensure_path()
    import concourse.bass as bass
    import concourse.mybir as mybir
    import concourse.tile as tile
    from concourse.masks import make_identity

    f32 = mybir.dt.float32
    fr = mybir.dt.float32r if use_f32r else f32
    nc = bass.Bass()

    xT = nc.declare_dram_parameter("xT", [D, BT], fr, isOutput=False)
    wqT = nc.declare_dram_parameter("wqT", [D, HPC * DH], fr, isOutput=False)
    wkT = nc.declare_dram_parameter("wkT", [D, DH], fr, isOutput=False)
    wvT = nc.declare_dram_parameter("wvT", [D, DH], fr, isOutput=False)
    woT = nc.declare_dram_parameter("woT", [HPC * DH, D], fr, isOutput=False)
    cosT = nc.declare_dram_parameter("cosT", [DH, T], f32, isOutput=False)
    rotMT = nc.declare_dram_parameter("rotMT", [DH, DH], fr, isOutput=False)
    sinT = nc.declare_dram_parameter("sinT", [DH, T], f32, isOutput=False)
    out = nc.declare_dram_parameter("out", [BT, D], f32, isOutput=True)

    with nc.allow_low_precision(reason="float32r fast matmul path"), \
         tile.TileContext(nc) as tc:
        with tc.tile_pool(name="persist", bufs=1) as P:
            ident = P.tile([128, 128], f32, tag="ident")
            maskT = P.tile([128, 128], f32, tag="maskT")
            ones = P.tile([128, 1], fr, tag="ones")
            ones_r = P.tile([1, 128], fr, tag="ones_r")
            ones_f = P.tile([128, 1], f32, tag="ones_f")
            ones_rf = P.tile([1, 128], f32, tag="ones_rf")
            make_identity(nc, ident[:])
            # S^T diag block mask: keep (s_local - t_local) <= 0, else -1e4
            nc.gpsimd.memset(maskT[:], 0.0)
            # keep where (t_local - s_local) >= 0, i.e. s <= t
            nc.gpsimd.affine_select(
                out=maskT[:],
                in_=maskT[:],
                compare_op=mybir.AluOpType.is_ge,
                fill=NEG,
                base=0,
                pattern=[[1, 128]],
                channel_multiplier=-1,
            )
            nc.gpsimd.memset(ones_f[:], 1.0)
            nc.gpsimd.memset(ones_rf[:], 1.0)
            nc.vector.tensor_copy(ones[:], ones_f[:])
            nc.vector.tensor_copy(ones_r[:], ones_rf[:])

            rotm_sb = P.tile([128, 128], fr, tag="rotm")
            cos_sb = P.tile([128, T], f32, tag="cos")
            sin_sb = P.tile([128, T], f32, tag="sin")
            QT = [P.tile([128, BT], fr, tag=f"qt{h}", name=f"qt{h}") for h in range(HPC)]
            KT = P.tile([128, BT], fr, tag="kt")
            VT = P.tile([128, BT], f32, tag="vt")
            Vn = P.tile([128, BT], fr, tag="vn")
            AVT = [P.tile([128, BT], fr, tag=f"avt{h}", name=f"avt{h}") for h in range(HPC)]

            # ---------- phase A: QKV projections + RoPE ----------
            with tc.tile_pool(name="wpool", bufs=1) as WP, \
                 tc.tile_pool(name="xp", bufs=3) as XP, \
                 tc.tile_pool(name="ropetA", bufs=4) as RT2, \
                 tc.tile_pool(name="psA", bufs=1, space="PSUM") as PSA, \
                 tc.tile_pool(name="psScrA", bufs=3, space="PSUM") as PSCR, \
                 tc.tile_pool(name="psWarm", bufs=1, space="PSUM") as PSW:
                wq_sb = WP.tile([128, ND * HPC * DH], fr, tag="wq")
                wk_sb = WP.tile([128, ND * DH], fr, tag="wk")
                wv_sb = WP.tile([128, ND * DH], fr, tag="wv")
                def load_weight_quarter(qi):
                    lo, hi = qi * (ND // 4), (qi + 1) * (ND // 4)
                    nc.sync.dma_start(
                        out=wq_sb[:, lo * 256: hi * 256].rearrange(
                            "p (c m) -> p c m", c=hi - lo),
                        in_=wqT[lo * 128: hi * 128, :].rearrange(
                            "(c p) m -> p c m", p=128))
                    nc.sync.dma_start(
                        out=wk_sb[:, lo * 128: hi * 128].rearrange(
                            "p (c m) -> p c m", c=hi - lo),
                        in_=wkT[lo * 128: hi * 128, :].rearrange(
                            "(c p) m -> p c m", p=128))
                    nc.sync.dma_start(
                        out=wv_sb[:, lo * 128: hi * 128].rearrange(
                            "p (c m) -> p c m", c=hi - lo),
                        in_=wvT[lo * 128: hi * 128, :].rearrange(
                            "(c p) m -> p c m", p=128))

                # keep-warm matmuls: the PE would otherwise sit idle for
                # ~8us of weight/x DMA and pay the HAM half-clock ramp when
                # the first projections finally issue. ~5us of dummy work on
                # the identity tile ramps the array to full speed for free.
                for _ in range(24):
                    wps = PSW.tile([128, 128], f32, tag="warm", name="warm")
                    nc.tensor.matmul(wps[:], lhsT=ident[:], rhs=ident[:],
                                     start=True, stop=True)

                # only the first quarter of the weights before the first x
                # tile; the rest interleave with tq5=0's x loads so the first
                # matmuls start ~12us earlier
                load_weight_quarter(0)

                for tq5 in range(BT // 512):
                    pq = [PSA.tile([128, 512], f32, tag=f"pq{h}", name=f"pq{h}") for h in range(HPC)]
                    pk = PSA.tile([128, 512], f32, tag="pk")
                    pv = PSA.tile([128, 512], f32, tag="pv")
                    for dcg in range(4):
                        # one 1MB DMA: 4 d-chunks x 512 tokens
                        xt = XP.tile([128, 4 * 512], fr, tag="x")
                        nc.sync.dma_start(
                            out=xt[:].rearrange("p (c m) -> p c m", c=4),
                            in_=xT[dcg * 512:(dcg + 1) * 512,
                                   tq5 * 512:(tq5 + 1) * 512].rearrange(
                                       "(c p) m -> p c m", p=128))
                        if tq5 == 0 and dcg < 3:
                            load_weight_quarter(dcg + 1)
                        if tq5 == 0 and dcg == 3:
                            # tables after all weights/first x tiles; ready
                            # well before the first inline rope needs them
                            nc.sync.dma_start(out=rotm_sb[:], in_=rotMT[:, :])
                            nc.sync.dma_start(out=cos_sb[:], in_=cosT[:, :])
                            nc.sync.dma_start(out=sin_sb[:], in_=sinT[:, :])
                        for dci in range(4):
                            dc = dcg * 4 + dci
                            xs = xt[:, dci * 512:(dci + 1) * 512]
                            st, sp = (dc == 0), (dc == ND - 1)
                            for h in range(HPC):
                                nc.tensor.matmul(
                                    pq[h][:],
                                    lhsT=wq_sb[:, dc * 256 + h * 128: dc * 256 + (h + 1) * 128],
                                    rhs=xs, start=st, stop=sp)
                            nc.tensor.matmul(
                                pk[:], lhsT=wk_sb[:, dc * 128:(dc + 1) * 128],
                                rhs=xs, start=st, stop=sp)
                            nc.tensor.matmul(
                                pv[:], lhsT=wv_sb[:, dc * 128:(dc + 1) * 128],
                                rhs=xs, start=st, stop=sp)
                    tcol = slice(tq5 * 512, (tq5 + 1) * 512)
                    for h in range(HPC):
                        nc.vector.tensor_copy(QT[h][:, tcol], pq[h][:, :])
                    nc.vector.tensor_copy(KT[:, tcol], pk[:, :])
                    nc.vector.tensor_copy(VT[:, tcol], pv[:, :])
                    # RoPE for this 512-token block, inline with projections
                    tab = slice((tq5 * 512) % T, (tq5 * 512) % T + 512)
                    for tgt in [KT, QT[0], QT[1]]:
                        rot_ps = PSCR.tile([128, 512], f32, tag="scr", name="rot_ps")
                        nc.tensor.matmul(rot_ps[:], lhsT=rotm_sb[:],
                                         rhs=tgt[:, tcol], start=True, stop=True)
                        rtmp = RT2.tile([128, 512], f32, tag="rtmp")
                        nc.vector.tensor_mul(rtmp[:], rot_ps[:], sin_sb[:, tab])
                        nc.vector.tensor_mul(tgt[:, tcol], tgt[:, tcol], cos_sb[:, tab])
                        nc.gpsimd.tensor_add(tgt[:, tcol], tgt[:, tcol], rtmp[:])
                    # V^T -> V natural for this block
                    for vc in range(tq5 * 4, (tq5 + 1) * 4):
                        vps = PSCR.tile([128, 128], f32, tag="scr", name="vtp")
                        nc.tensor.transpose(vps[:], VT[:, vc * 128:(vc + 1) * 128],
                                            ident[:])
                        nc.vector.tensor_copy(Vn[:, vc * 128:(vc + 1) * 128], vps[:])
            # ---------- phases A2/B0/B/C merged: rope, V-transpose,
            # attention, and interleaved output projection in one scope ------
            with tc.tile_pool(name="wop", bufs=1) as WOP, \
                 tc.tile_pool(name="ptp", bufs=4) as PTP, \
                 tc.tile_pool(name="rrp", bufs=2) as RRP, \
                 tc.tile_pool(name="osbp", bufs=3) as OSBP, \
                 tc.tile_pool(name="psSt", bufs=4, space="PSUM") as PSST, \
                 tc.tile_pool(name="psL", bufs=1, space="PSUM") as PSL, \
                 tc.tile_pool(name="psAv", bufs=1, space="PSUM") as PSAV, \
                 tc.tile_pool(name="psC", bufs=1, space="PSUM") as PSC:
                wo_sb = WOP.tile([128, HPC * D], fr, tag="wo")
                nc.sync.dma_start(
                    out=wo_sb[:].rearrange("p (c n) -> p c n", c=HPC),
                    in_=woT[:, :].rearrange("(c p) n -> p c n", p=128))
                # attention + output projection
                # keep the longest group (tsb=3) off the tail position
                groups = [(0, 0), (0, 1), (0, 2), (0, 3),
                          (1, 0), (1, 3), (1, 1), (1, 2)]
                for b, tsb in groups:
                    if True:
                        n_sc = (tsb + 1) * 4
                        tg = slice(b * T + tsb * 512, b * T + (tsb + 1) * 512)
                        for h in range(HPC):
                            av_ps = PSAV.tile([128, 512], f32, tag="av")
                            l_ps = PSL.tile([1, 512], f32, tag="l")
                            for sc in range(n_sc):
                                sc_rel = sc - tsb * 4
                                c0 = max(sc_rel, 0) * 128  # first valid t col
                                nv = slice(c0, 512)
                                tgn = slice(b * T + tsb * 512 + c0,
                                            b * T + (tsb + 1) * 512)
                                st_ps = PSST.tile([128, 512], f32, tag="st")
                                nc.tensor.matmul(
                                    st_ps[:, nv],
                                    lhsT=KT[:, b * T + sc * 128: b * T + (sc + 1) * 128],
                                    rhs=QT[h][:, tgn], start=True, stop=True)
                                if sc_rel >= 0:
                                    blk = st_ps[:, c0:c0 + 128]
                                    nc.vector.tensor_add(blk, blk, maskT[:])
                                pt = PTP.tile([128, 512], fr, tag="pt")
                                nc.scalar.activation(
                                    pt[:, nv], st_ps[:, nv],
                                    mybir.ActivationFunctionType.Exp)
                                nc.tensor.matmul(
                                    l_ps[:, nv], lhsT=ones[:], rhs=pt[:, nv],
                                    start=(sc == 0), stop=(sc == n_sc - 1))
                                nc.tensor.matmul(
                                    av_ps[:, nv],
                                    lhsT=Vn[:, b * T + sc * 128: b * T + (sc + 1) * 128],
                                    rhs=pt[:, nv], start=(sc == 0), stop=(sc == n_sc - 1))
                            rr = RRP.tile([1, 512], fr, tag="rr")
                            nc.vector.reciprocal(rr[:], l_ps[:])
                            nc.vector.tensor_copy(AVT[h][:, tg], av_ps[:])
                            rbc = PSC.tile([128, 512], f32, tag="wops", name="rbc")
                            nc.tensor.matmul(
                                rbc[:], lhsT=ones_r[:], rhs=rr[:],
                                start=True, stop=True)
                            nc.vector.tensor_mul(AVT[h][:, tg], AVT[h][:, tg], rbc[:])
                        # both heads done for this 512-token group: project out
                        for tj in range(4):
                            tcx = (b * T + tsb * 512) // 128 + tj
                            for dhalf in range(2):
                                wo_ps = PSC.tile([128, D // 2], f32, tag="wops")
                                for h in range(HPC):
                                    for ndc in range(2):
                                        ns = slice(ndc * 512, (ndc + 1) * 512)
                                        nc.tensor.matmul(
                                            wo_ps[:, ns],
                                            lhsT=AVT[h][:, tcx * 128:(tcx + 1) * 128],
                                            rhs=wo_sb[:, h * D + dhalf * 1024 + ndc * 512:
                                                      h * D + dhalf * 1024 + (ndc + 1) * 512],
                                            start=(h == 0), stop=(h == HPC - 1))
                                osb = OSBP.tile([128, D // 2], f32, tag="osb")
                                if (tj + dhalf) % 2 == 0:
                                    nc.vector.tensor_copy(osb[:], wo_ps[:])
                                else:
                                    nc.scalar.copy(osb[:], wo_ps[:])
                                nc.sync.dma_start(
                                    out=out[tcx * 128:(tcx + 1) * 128,
                                            dhalf * 1024:(dhalf + 1) * 1024],
                                    in_=osb[:])

    if split_waits:
        _split_multi_waits(nc, mybir)
    return nc


def _host_inputs(x, wq, wk, wv, wo):
    xT = np.ascontiguousarray(x.reshape(BT, D).T)
    half = DH // 2
    inv = (1.0 / (ROPE_BASE ** (np.arange(half, dtype=np.float32) / half))).astype(np.float32)
    ang = np.arange(T, dtype=np.float32)[:, None] * inv[None, :]          # (T, 64)
    c = np.cos(ang).T.astype(np.float32)                                  # (64, T)
    s = np.sin(ang).T.astype(np.float32)
    cosT = np.ascontiguousarray(np.concatenate([c, c], axis=0))           # (128, T)
    sinT = np.ascontiguousarray(np.concatenate([s, s], axis=0))
    rotMT = np.zeros((DH, DH), dtype=np.float32)
    rotMT[np.arange(64), np.arange(64) + 64] = 1.0    # lhsT: rotM[i+64, i] ... rot = rotM @ q
    rotMT[np.arange(64) + 64, np.arange(64)] = -1.0
    scale = np.float32(1.0 / np.sqrt(DH))
    in_maps = []
    for core in range(NCORES):
        kvh = core // 2
        in_maps.append({
            "xT": xT,
            "wqT": np.ascontiguousarray((wq[core * HPC * DH:(core + 1) * HPC * DH, :] * scale).T),
            "wkT": np.ascontiguousarray(wk[kvh * DH:(kvh + 1) * DH, :].T),
            "wvT": np.ascontiguousarray(wv[kvh * DH:(kvh + 1) * DH, :].T),
            "woT": np.ascontiguousarray(wo[:, core * HPC * DH:(core + 1) * HPC * DH].T),
            "cosT": cosT,
            "sinT": sinT,
            "rotMT": rotMT,
        })
    return in_maps


def kernel(x, wq, wk, wv, wo):
    _ensure_path()
    from concourse.bass_utils import run_bass_kernel_spmd

    x = np.asarray(x, dtype=np.float32)
    wq = np.asarray(wq, dtype=np.float32)
    wk = np.asarray(wk, dtype=np.float32)
    wv = np.asarray(wv, dtype=np.float32)
    wo = np.asarray(wo, dtype=np.float32)

    if "nc" not in _cache:
        _cache["nc"] = _build()
    nc = _cache["nc"]

    in_maps = _host_inputs(x, wq, wk, wv, wo)
    res = run_bass_kernel_spmd(nc, in_maps, list(range(NCORES)))
    acc = res.results[0]["out"].astype(np.float32)
    for cidx in range(1, NCORES):
        acc = acc + res.results[cidx]["out"]
    return acc.reshape(B, T, D)



# revision 40
# speedup vs baseline: 1.2582x; 1.2582x over previous
"""Tensor-parallel GQA multi-head attention for 8 Trainium2 NeuronCores.

Sharding: query heads (16) split 2-per-core; each core needs exactly one
KV head (GQA group); wq/wk/wv column-parallel, wo row-parallel; the
all-reduce after wo is done host-side (sum of 8 partial outputs).

Per-core dataflow (all matmul operands bf16, PSUM f32):
  QT/KT = Wqk^T-chunks (lhsT) x xT (rhs)       [dh, tokens]
  Vn    = xT-chunks (lhsT) x Wv^T (rhs)        [tokens, dh] natural layout
  rope  on QT/KT via partition-shifted DVE muls (no PE rotation matmul)
  S^T   = KT-chunk (lhsT) x QT (rhs)           [s, t]  (causal superblocks,
                                                paired into 1024-wide PSUM
                                                super-tiles for cheaper exp)
  P^T   = exp(S^T + causal mask)               (no max-subtraction: scores
                                                are bounded ~N(0, 1/9))
  l     = Pool-accumulated sum of P^T chunks, then one ones-matmul  [1, t]
  avT   = Vn-chunk (lhsT) x P^T (rhs)          [dh, t]; scaled by 1/l
  out   = avT-chunk (lhsT) x woT (rhs)         [t, d] partial, bf16 DMA out

Q/K/V and AV live in per-512-token-superblock tiles so the Tile
framework's whole-tile dependency tracking doesn't serialize attention
behind the last projection. Each group's output projection is deferred
by one group so the l->recip->broadcast->scale chain of the current
group hides under the previous group's wo matmuls.
"""

import numpy as np

B, T, D, H, KV = 2, 2048, 2048, 16, 4
DH = 128
NCORES = 8
HPC = H // NCORES          # 2 query heads per core
BT = B * T                 # 4096
ND = D // 128              # 16 contraction chunks
NSB = T // 512             # 4 causal superblocks per batch
NG = B * NSB               # 8 (batch, superblock) groups
ROPE_BASE = 10000.0
NEG = -1.0e4
_WO_RESERVE = 8            # wo units kept for the group-end flush
_N_WARM = 8                # keep-warm matmuls before the first projection

_cache = {}


def _ensure_path():
    try:
        import concourse.bass  # noqa: F401
    except ImportError:
        import sys
        for p in ("/opt/trn_rl_repo", "/root/.axon_site/_ro/trn_rl_repo"):
            if p not in sys.path:
                sys.path.insert(0, p)
        import concourse.bass  # noqa: F401


def _split_multi_waits(nc, mybir, max_waits=1):
    """This container's walrus rejects >1 sync-wait on one instruction
    (seen on the Tile tail drain). Move extra waits onto preceding NoOps
    on the same engine; per-engine program order preserves semantics."""
    for bb in nc.main_func.blocks:
        new_insts = []
        for ins in bb.instructions:
            si = getattr(ins, "sync_info", None)
            if si is not None and si.on_wait and len(si.on_wait) > max_waits:
                waits = list(si.on_wait)
                extra, keep = waits[:-max_waits], waits[-max_waits:]
                for w in extra:
                    new_insts.append(
                        mybir.InstNoOp(
                            name=nc.get_next_instruction_name(),
                            sync_info=mybir.SyncInfo(on_wait=[w], on_update=[]),
                            bass_nofuse=True,
                            engine=ins.engine,
                            ins=[],
                            outs=[],
                        )
                    )
                si.on_wait = keep
            new_insts.append(ins)
        bb.instructions = new_insts


def _build(split_waits=True):
    _ensure_path()
    import concourse.bass as bass
    import concourse.mybir as mybir
    import concourse.tile as tile
    from concourse.masks import make_identity

    f32 = mybir.dt.float32
    fr = mybir.dt.float32r
    bf16 = mybir.dt.bfloat16
    nc = bass.Bass()

    xT = nc.declare_dram_parameter("xT", [D, BT], bf16, isOutput=False)
    # per 128-row chunk c: cols 0:128 q head0, 128:256 q head1, 256:384 k,
    # 384:512 v
    wqkvT = nc.declare_dram_parameter("wqkvT", [D, 4 * DH], bf16, isOutput=False)
    woT = nc.declare_dram_parameter("woT", [HPC * DH, D], bf16, isOutput=False)
    cosT = nc.declare_dram_parameter("cosT", [DH, T], bf16, isOutput=False)
    # rows 0:64 hold +sin, rows 64:128 hold -sin
    nsinT = nc.declare_dram_parameter("nsinT", [DH, T], bf16, isOutput=False)
    out = nc.declare_dram_parameter("out", [BT, D], bf16, isOutput=True)

    with nc.allow_low_precision(reason="bf16 fast matmul path"), \
         tile.TileContext(nc) as tc:
        with tc.tile_pool(name="persist", bufs=1) as P:
            ident = P.tile([128, 128], f32, tag="ident")
            ones_sq = P.tile([128, 128], fr, tag="ones_sq")
            ones_sqf = P.tile([128, 128], f32, tag="ones_sqf")
            ones_rf = P.tile([1, 128], f32, tag="ones_rf")
            make_identity(nc, ident[:])
            nc.gpsimd.memset(ones_sqf[:], 1.0)
            nc.vector.tensor_copy(ones_sq[:], ones_sqf[:])
            nc.gpsimd.memset(ones_rf[:], 1.0)
            # dummy exp: pull the 1.3us activation-table load into phase A
            # instead of paying it on the first real softmax exp
            dummy_e = P.tile([1, 1], f32, tag="dummy_e")
            nc.scalar.activation(dummy_e[:], ones_rf[0:1, 0:1],
                                 mybir.ActivationFunctionType.Exp)

            cos_sb = P.tile([128, T], bf16, tag="cos")
            nsin_sb = P.tile([128, T], bf16, tag="nsin")
            wo_sb = P.tile([128, HPC * D], bf16, tag="wo")
            # per-superblock tiles: fine-grained deps let attention start
            # on a superblock as soon as its projections+rope finish
            QTs = [[P.tile([128, 512], bf16, tag=f"qt{h}_{g}", name=f"qt{h}_{g}")
                    for g in range(NG)] for h in range(HPC)]
            KTs = [P.tile([128, 512], bf16, tag=f"kt{g}", name=f"kt{g}")
                   for g in range(NG)]
            Vns = [P.tile([128, 512], bf16, tag=f"vn{g}", name=f"vn{g}")
                   for g in range(NG)]
            AVTs = [[P.tile([128, 512], bf16, tag=f"avt{h}_{g}", name=f"avt{h}_{g}")
                     for g in range(NG)] for h in range(HPC)]

            # ---------- phase A: QKV projections + RoPE ----------
            with tc.tile_pool(name="wpool", bufs=1) as WP, \
                 tc.tile_pool(name="xp", bufs=4) as XP, \
                 tc.tile_pool(name="ropetA", bufs=4) as RT2, \
                 tc.tile_pool(name="psA", bufs=1, space="PSUM") as PSA, \
                 tc.tile_pool(name="psWarm", bufs=1, space="PSUM") as PSW:
                w_sb = WP.tile([128, ND * 4 * DH], bf16, tag="w")
                def load_weight_quarter(qi):
                    lo, hi = qi * (ND // 4), (qi + 1) * (ND // 4)
                    nc.sync.dma_start(
                        out=w_sb[:, lo * 512: hi * 512].rearrange(
                            "p (c m) -> p c m", c=hi - lo),
                        in_=wqkvT[lo * 128: hi * 128, :].rearrange(
                            "(c p) m -> p c m", p=128))

                # keep-warm matmuls: the PE would otherwise sit idle for
                # the initial weight/x DMA and pay the HAM half-clock ramp
                # when the first projections finally issue.
                for _ in range(_N_WARM):
                    wps = PSW.tile([128, 128], f32, tag="warm", name="warm")
                    nc.tensor.matmul(wps[:], lhsT=ident[:], rhs=ident[:],
                                     start=True, stop=True)

                # only the first quarter of the weights before the first x
                # tile; the rest interleave with tq5=0's x loads so the first
                # matmuls start earlier
                load_weight_quarter(0)

                for tq5 in range(NG):
                    pq = [PSA.tile([128, 512], f32, tag=f"pq{h}", name=f"pq{h}") for h in range(HPC)]
                    pk = PSA.tile([128, 512], f32, tag="pk")
                    pv = [PSA.tile([128, 128], f32, tag=f"pv{tb}", name=f"pv{tb}") for tb in range(4)]
                    for dcg in range(4):
                        # one 512KB DMA: 4 d-chunks x 512 tokens
                        xt = XP.tile([128, 4 * 512], bf16, tag="x")
                        nc.sync.dma_start(
                            out=xt[:].rearrange("p (c m) -> p c m", c=4),
                            in_=xT[dcg * 512:(dcg + 1) * 512,
                                   tq5 * 512:(tq5 + 1) * 512].rearrange(
                                       "(c p) m -> p c m", p=128))
                        if tq5 == 0 and dcg < 3:
                            load_weight_quarter(dcg + 1)
                        # tables + wo spread through tq5=1/2 x loads: the
                        # first rope (DVE) has ~100us of slack before its
                        # consumer, so keep the DMA queue clear for x tiles
                        if tq5 == 1 and dcg == 1:
                            nc.sync.dma_start(out=cos_sb[:], in_=cosT[:, :])
                        if tq5 == 1 and dcg == 3:
                            nc.sync.dma_start(out=nsin_sb[:], in_=nsinT[:, :])
                        if tq5 == 2 and dcg in (1, 3):
                            wh = dcg // 2
                            nc.sync.dma_start(
                                out=wo_sb[:, wh * D:(wh + 1) * D],
                                in_=woT[wh * 128:(wh + 1) * 128, :])
                        for dci in range(4):
                            dc = dcg * 4 + dci
                            xs = xt[:, dci * 512:(dci + 1) * 512]
                            st, sp = (dc == 0), (dc == ND - 1)
                            for h in range(HPC):
                                nc.tensor.matmul(
                                    pq[h][:],
                                    lhsT=w_sb[:, dc * 512 + h * 128: dc * 512 + (h + 1) * 128],
                                    rhs=xs, start=st, stop=sp)
                            nc.tensor.matmul(
                                pk[:], lhsT=w_sb[:, dc * 512 + 256: dc * 512 + 384],
                                rhs=xs, start=st, stop=sp)
                            # V in natural [token, dh] layout: x-chunk as lhsT
                            for tb in range(4):
                                nc.tensor.matmul(
                                    pv[tb][:],
                                    lhsT=xs[:, tb * 128:(tb + 1) * 128],
                                    rhs=w_sb[:, dc * 512 + 384: dc * 512 + 512],
                                    start=st, stop=sp)
                    # Vn copies first (Pool cannot read PSUM on real HW) so
                    # the pv banks drain promptly for the attention st tiles
                    for tb in range(4):
                        nc.vector.tensor_copy(
                            Vns[tq5][:, tb * 128:(tb + 1) * 128], pv[tb][:, :])
                    for h in range(HPC):
                        nc.vector.tensor_copy(QTs[h][tq5][:, :], pq[h][:, :])
                    nc.vector.tensor_copy(KTs[tq5][:, :], pk[:, :])
                    # RoPE is deferred one tq5 iteration: its cos/nsin
                    # tables are DMA'd during tq5=1, and the tile framework
                    # orders deps by emission, so rope(k) is emitted at the
                    # end of iteration k+1 (consumers are ~100us away).
                    # HW cannot partition-shift in a 2-input op, but a
                    # 1-input DVE copy can: multiply by nsin in place
                    # (rows 0:64 = +sin, 64:128 = -sin), then swap halves
                    # with two shifted copies to build rot*sin.
                    def emit_rope(idx):
                        tab = slice((idx * 512) % T, (idx * 512) % T + 512)
                        for tgt in [KTs[idx], QTs[0][idx], QTs[1][idx]]:
                            rtmp = RT2.tile([128, 512], bf16, tag="rtmp")
                            nc.vector.tensor_mul(rtmp[:], tgt[:, :],
                                                 nsin_sb[:, tab])
                            rtsw = RT2.tile([128, 512], bf16, tag="rtsw")
                            nc.vector.tensor_copy(rtsw[0:64, :], rtmp[64:128, :])
                            nc.vector.tensor_copy(rtsw[64:128, :], rtmp[0:64, :])
                            nc.vector.tensor_mul(tgt[:, :], tgt[:, :],
                                                 cos_sb[:, tab])
                            nc.gpsimd.tensor_add(tgt[:, :], tgt[:, :], rtsw[:])
                    if tq5 >= 1:
                        emit_rope(tq5 - 1)
                    if tq5 == NG - 1:
                        emit_rope(tq5)
            # ---------- attention + deferred output projection ----------
            # PSUM budget (16KB/partition): st 2x4KB + av 2KB + l/rbc 2KB
            # + wo 2x2KB = 16KB
            with tc.tile_pool(name="ptp", bufs=6) as PTP, \
                 tc.tile_pool(name="lap", bufs=3) as LAP, \
                 tc.tile_pool(name="rrp", bufs=2) as RRP, \
                 tc.tile_pool(name="osbp", bufs=4) as OSBP, \
                 tc.tile_pool(name="psAv", bufs=1, space="PSUM") as PSAV, \
                 tc.tile_pool(name="psL", bufs=1, space="PSUM") as PSL, \
                 tc.tile_pool(name="psC", bufs=2, space="PSUM") as PSC, \
                 tc.tile_pool(name="psSt", bufs=2, space="PSUM") as PSST:
                # prime PSUM slot allocation order so the st super-tiles
                # land on the banks that held pv0-3 (drained early on Pool)
                # rather than the pq/pk banks that drain last on DVE
                PSAV.tile([128, 512], f32, tag="av", name="prime_av")
                PSL.tile([128, 512], f32, tag="lr", name="prime_lr")
                PSC.tile([128, 512], f32, tag="wops", name="prime_wops")
                # also rotate the st pool: the first real st tile then takes
                # buf1 (fully free banks) while buf0 overlaps the still-
                # draining pv banks from the last projection block
                PSST.tile([128, 1024], f32, tag="st", name="prime_st")
                osb_cnt = [0]

                def wo_units(b, tsb, fine=False):
                    # output projection for one 512-token group as 8
                    # independently emittable units (one osb half each),
                    # 512-wide d-quarters with double-buffered PSUM.
                    # fine=True DMAs each quarter separately (shorter tail).
                    g = b * NSB + tsb
                    units = []
                    for tj in range(4):
                        for dhalf in range(2):
                            def unit(tj=tj, dhalf=dhalf):
                                tcx = (b * T + tsb * 512) // 128 + tj
                                osb = OSBP.tile([128, D // 2], bf16, tag="osb")
                                for dq in range(2):
                                    dcol = dhalf * 1024 + dq * 512
                                    wo_ps = PSC.tile([128, 512], f32, tag="wops")
                                    for h in range(HPC):
                                        nc.tensor.matmul(
                                            wo_ps[:],
                                            lhsT=AVTs[h][g][:, tj * 128:(tj + 1) * 128],
                                            rhs=wo_sb[:, h * D + dcol: h * D + dcol + 512],
                                            start=(h == 0), stop=(h == HPC - 1))
                                    # only DVE and Act can read PSUM; Act
                                    # also runs exp, so weight DVE 2:1
                                    eng = osb_cnt[0] % 3
                                    osb_cnt[0] += 1
                                    dst = osb[:, dq * 512:(dq + 1) * 512]
                                    if eng == 2:
                                        nc.scalar.copy(dst, wo_ps[:])
                                    else:
                                        nc.vector.tensor_copy(dst, wo_ps[:])
                                    if fine:
                                        nc.sync.dma_start(
                                            out=out[tcx * 128:(tcx + 1) * 128,
                                                    dcol:dcol + 512],
                                            in_=dst)
                                if not fine:
                                    nc.sync.dma_start(
                                        out=out[tcx * 128:(tcx + 1) * 128,
                                                dhalf * 1024:(dhalf + 1) * 1024],
                                        in_=osb[:])
                            units.append(unit)
                    return units

                # longest groups first: more full pairs early keeps the PE
                # ahead of the exp chain while there is no wo fill yet, and
                # the shortest group lands on the tail
                groups = [(0, 3), (0, 2), (0, 1), (0, 0),
                          (1, 3), (1, 2), (1, 1), (1, 0)]
                pending = []
                for b, tsb in groups:
                    n_sc = (tsb + 1) * 4
                    g = b * NSB + tsb
                    # interleave some of the previous group's wo units into
                    # this group's pair loop to fill exp-chain stalls; the
                    # rest stay for the group end to hide the l->recip->
                    # scale chain
                    drip_budget = [max(0, len(pending) - _WO_RESERVE)]
                    def drip():
                        if drip_budget[0] > 0 and pending:
                            pending.pop(0)()
                            drip_budget[0] -= 1
                    for h in range(HPC):
                        av_ps = PSAV.tile([128, 512], f32, tag="av")
                        pacc = LAP.tile([128, 512], fr, tag="pacc")
                        for scp in range(n_sc // 2):
                            st2 = PSST.tile([128, 1024], f32, tag="st")
                            pt2 = PTP.tile([128, 1024], bf16, tag="pt")
                            halves = []
                            for k in (0, 1):
                                sc = 2 * scp + k
                                c0 = max(sc - tsb * 4, 0) * 128
                                off = k * 512
                                nc.tensor.matmul(
                                    st2[:, off + c0: off + 512],
                                    lhsT=KTs[b * NSB + sc // 4][:, (sc % 4) * 128:
                                                                (sc % 4 + 1) * 128],
                                    rhs=QTs[h][g][:, c0:512],
                                    start=True, stop=True)
                                halves.append((sc, c0, off))
                            if halves[1][1] == 0:
                                # both halves full-width: one paired exp
                                nc.scalar.activation(
                                    pt2[:, :], st2[:, :],
                                    mybir.ActivationFunctionType.Exp)
                            else:
                                for sc, c0, off in halves:
                                    nc.scalar.activation(
                                        pt2[:, off + c0: off + 512],
                                        st2[:, off + c0: off + 512],
                                        mybir.ActivationFunctionType.Exp)
                            for sc, c0, off in halves:
                                # causal mask: zero the upper triangle of
                                # the diagonal 128-block post-exp on Pool
                                # (keeps DVE free; exp of unmasked scores
                                # is bounded, ~exp(1.5) max)
                                if sc >= tsb * 4:
                                    nc.gpsimd.affine_select(
                                        out=pt2[:, off + c0: off + c0 + 128],
                                        in_=pt2[:, off + c0: off + c0 + 128],
                                        compare_op=mybir.AluOpType.is_ge,
                                        fill=0.0,
                                        base=0,
                                        pattern=[[1, 128]],
                                        channel_multiplier=-1,
                                    )
                                # softmax denominators accumulate on Pool
                                # (saves one PE column-stream per chunk)
                                if sc == 0:
                                    nc.gpsimd.tensor_copy(pacc[:, :], pt2[:, 0:512])
                                else:
                                    nc.gpsimd.tensor_add(
                                        pacc[:, c0:512], pacc[:, c0:512],
                                        pt2[:, off + c0: off + 512])
                                nc.tensor.matmul(
                                    av_ps[:, c0:512],
                                    lhsT=Vns[b * NSB + sc // 4][:, (sc % 4) * 128:
                                                                (sc % 4 + 1) * 128],
                                    rhs=pt2[:, off + c0: off + 512],
                                    start=(sc == 0), stop=(sc == n_sc - 1))
                            drip()
                        # copy first: frees av_ps for the next head's
                        # accumulation while the l chain runs
                        nc.vector.tensor_copy(AVTs[h][g][:, :], av_ps[:])
                        # all-ones lhsT: one matmul both sums pacc over s
                        # AND broadcasts l to every partition
                        l_bc = PSL.tile([128, 512], f32, tag="lr", name="l_bc")
                        nc.tensor.matmul(l_bc[:], lhsT=ones_sq[:], rhs=pacc[:],
                                         start=True, stop=True)
                        linv = RRP.tile([128, 512], bf16, tag="linv")
                        nc.vector.reciprocal(linv[:], l_bc[:])
                        nc.vector.tensor_mul(AVTs[h][g][:, :], AVTs[h][g][:, :],
                                             linv[:])
                    while pending:
                        pending.pop(0)()
                    pending = wo_units(b, tsb, fine=(b, tsb) == groups[-1])
                for u in pending:
                    u()

    if split_waits:
        _split_multi_waits(nc, mybir)
    return nc


def _host_inputs(x, wq, wk, wv, wo):
    import ml_dtypes
    bf = ml_dtypes.bfloat16
    xT = np.ascontiguousarray(x.reshape(BT, D).T.astype(bf))
    half = DH // 2
    inv = (1.0 / (ROPE_BASE ** (np.arange(half, dtype=np.float32) / half))).astype(np.float32)
    ang = np.arange(T, dtype=np.float32)[:, None] * inv[None, :]          # (T, 64)
    c = np.cos(ang).T.astype(np.float32)                                  # (64, T)
    s = np.sin(ang).T.astype(np.float32)
    cosT = np.ascontiguousarray(np.concatenate([c, c], axis=0).astype(bf))  # (128, T)
    nsinT = np.ascontiguousarray(np.concatenate([s, -s], axis=0).astype(bf))
    scale = np.float32(1.0 / np.sqrt(DH))
    in_maps = []
    for core in range(NCORES):
        kvh = core // 2
        wqkvT = np.concatenate([
            (wq[core * HPC * DH:(core + 1) * HPC * DH, :] * scale).T,
            wk[kvh * DH:(kvh + 1) * DH, :].T,
            wv[kvh * DH:(kvh + 1) * DH, :].T,
        ], axis=1)
        in_maps.append({
            "xT": xT,
            "wqkvT": np.ascontiguousarray(wqkvT.astype(bf)),
            "woT": np.ascontiguousarray(wo[:, core * HPC * DH:(core + 1) * HPC * DH].T.astype(bf)),
            "cosT": cosT,
            "nsinT": nsinT,
        })
    return in_maps


def kernel(x, wq, wk, wv, wo):
    _ensure_path()
    from concourse.bass_utils import run_bass_kernel_spmd

    x = np.asarray(x, dtype=np.float32)
    wq = np.asarray(wq, dtype=np.float32)
    wk = np.asarray(wk, dtype=np.float32)
    wv = np.asarray(wv, dtype=np.float32)
    wo = np.asarray(wo, dtype=np.float32)

    if "nc" not in _cache:
        _cache["nc"] = _build()
    nc = _cache["nc"]

    in_maps = _host_inputs(x, wq, wk, wv, wo)
    res = run_bass_kernel_spmd(nc, in_maps, list(range(NCORES)))
    acc = np.asarray(res.results[0]["out"]).astype(np.float32)
    for cidx in range(1, NCORES):
        acc = acc + np.asarray(res.results[cidx]["out"]).astype(np.float32)
    return acc.reshape(B, T, D)


# revision 42
# speedup vs baseline: 1.2986x; 1.0321x over previous
"""Tensor-parallel GQA multi-head attention for 8 Trainium2 NeuronCores.

Sharding: query heads (16) split 2-per-core; each core needs exactly one
KV head (GQA group); wq/wk/wv column-parallel, wo row-parallel; the
all-reduce after wo is done host-side (sum of 8 partial outputs).

Per-core dataflow (all matmul operands bf16, PSUM f32):
  QT/KT = Wqk^T-chunks (lhsT) x xT (rhs)       [dh, tokens]
  Vn    = xT-chunks (lhsT) x Wv^T (rhs)        [tokens, dh] natural layout
  rope  on QT/KT via partition-shifted DVE muls (no PE rotation matmul)
  S^T   = KT-chunk (lhsT) x QT (rhs)           [s, t]  (causal superblocks,
                                                paired into 1024-wide PSUM
                                                super-tiles for cheaper exp)
  P^T   = exp(S^T + causal mask)               (no max-subtraction: scores
                                                are bounded ~N(0, 1/9))
  l     = Pool-accumulated sum of P^T chunks, then one ones-matmul  [1, t]
  avT   = Vn-chunk (lhsT) x P^T (rhs)          [dh, t]; scaled by 1/l
  out   = avT-chunk (lhsT) x woT (rhs)         [t, d] partial, bf16 DMA out

Q/K/V and AV live in per-512-token-superblock tiles so the Tile
framework's whole-tile dependency tracking doesn't serialize attention
behind the last projection. Each group's output projection is deferred
by one group so the l->recip->broadcast->scale chain of the current
group hides under the previous group's wo matmuls.
"""

import numpy as np

B, T, D, H, KV = 2, 2048, 2048, 16, 4
DH = 128
NCORES = 8
HPC = H // NCORES          # 2 query heads per core
BT = B * T                 # 4096
ND = D // 128              # 16 contraction chunks
NSB = T // 512             # 4 causal superblocks per batch
NG = B * NSB               # 8 (batch, superblock) groups
ROPE_BASE = 10000.0
NEG = -1.0e4
_WO_RESERVE = 8            # wo units kept for the group-end flush
_N_WARM = 8                # keep-warm matmuls before the first projection

_cache = {}


def _ensure_path():
    try:
        import concourse.bass  # noqa: F401
    except ImportError:
        import sys
        for p in ("/opt/trn_rl_repo", "/root/.axon_site/_ro/trn_rl_repo"):
            if p not in sys.path:
                sys.path.insert(0, p)
        import concourse.bass  # noqa: F401


def _split_multi_waits(nc, mybir, max_waits=1):
    """This container's walrus rejects >1 sync-wait on one instruction
    (seen on the Tile tail drain). Move extra waits onto preceding NoOps
    on the same engine; per-engine program order preserves semantics."""
    for bb in nc.main_func.blocks:
        new_insts = []
        for ins in bb.instructions:
            si = getattr(ins, "sync_info", None)
            if si is not None and si.on_wait and len(si.on_wait) > max_waits:
                waits = list(si.on_wait)
                extra, keep = waits[:-max_waits], waits[-max_waits:]
                for w in extra:
                    new_insts.append(
                        mybir.InstNoOp(
                            name=nc.get_next_instruction_name(),
                            sync_info=mybir.SyncInfo(on_wait=[w], on_update=[]),
                            bass_nofuse=True,
                            engine=ins.engine,
                            ins=[],
                            outs=[],
                        )
                    )
                si.on_wait = keep
            new_insts.append(ins)
        bb.instructions = new_insts


def _build(split_waits=True):
    _ensure_path()
    import concourse.bass as bass
    import concourse.mybir as mybir
    import concourse.tile as tile
    from concourse.masks import make_identity

    f32 = mybir.dt.float32
    fr = mybir.dt.float32r
    bf16 = mybir.dt.bfloat16
    nc = bass.Bass()

    xT = nc.declare_dram_parameter("xT", [D, BT], bf16, isOutput=False)
    # per 128-row chunk c: cols 0:128 q head0, 128:256 q head1, 256:384 k,
    # 384:512 v
    wqkvT = nc.declare_dram_parameter("wqkvT", [D, 4 * DH], bf16, isOutput=False)
    woT = nc.declare_dram_parameter("woT", [HPC * DH, D], bf16, isOutput=False)
    cosT = nc.declare_dram_parameter("cosT", [DH, T], bf16, isOutput=False)
    # rows 0:64 hold +sin, rows 64:128 hold -sin
    nsinT = nc.declare_dram_parameter("nsinT", [DH, T], bf16, isOutput=False)
    out = nc.declare_dram_parameter("out", [BT, D], bf16, isOutput=True)

    with nc.allow_low_precision(reason="bf16 fast matmul path"), \
         tile.TileContext(nc) as tc:
        with tc.tile_pool(name="persist", bufs=1) as P:
            ident = P.tile([128, 128], f32, tag="ident")
            ones_sq = P.tile([128, 128], fr, tag="ones_sq")
            ones_sqf = P.tile([128, 128], f32, tag="ones_sqf")
            ones_rf = P.tile([1, 128], f32, tag="ones_rf")
            make_identity(nc, ident[:])
            nc.gpsimd.memset(ones_sqf[:], 1.0)
            nc.vector.tensor_copy(ones_sq[:], ones_sqf[:])
            nc.gpsimd.memset(ones_rf[:], 1.0)
            # dummy exp: pull the 1.3us activation-table load into phase A
            # instead of paying it on the first real softmax exp
            dummy_e = P.tile([1, 1], f32, tag="dummy_e")
            nc.scalar.activation(dummy_e[:], ones_rf[0:1, 0:1],
                                 mybir.ActivationFunctionType.Exp)

            cos_sb = P.tile([128, T], bf16, tag="cos")
            nsin_sb = P.tile([128, T], bf16, tag="nsin")
            wo_sb = P.tile([128, HPC * D], bf16, tag="wo")
            # per-superblock tiles: fine-grained deps let attention start
            # on a superblock as soon as its projections+rope finish
            QTs = [[P.tile([128, 512], bf16, tag=f"qt{h}_{g}", name=f"qt{h}_{g}")
                    for g in range(NG)] for h in range(HPC)]
            KTs = [P.tile([128, 512], bf16, tag=f"kt{g}", name=f"kt{g}")
                   for g in range(NG)]
            Vns = [P.tile([128, 512], bf16, tag=f"vn{g}", name=f"vn{g}")
                   for g in range(NG)]
            AVTs = [[P.tile([128, 512], bf16, tag=f"avt{h}_{g}", name=f"avt{h}_{g}")
                     for g in range(NG)] for h in range(HPC)]

            # ---------- phase A: QKV projections + RoPE ----------
            with tc.tile_pool(name="wpool", bufs=1) as WP, \
                 tc.tile_pool(name="xp", bufs=4) as XP, \
                 tc.tile_pool(name="ropetA", bufs=4) as RT2, \
                 tc.tile_pool(name="psA", bufs=1, space="PSUM") as PSA, \
                 tc.tile_pool(name="psWarm", bufs=1, space="PSUM") as PSW:
                w_sb = WP.tile([128, ND * 4 * DH], bf16, tag="w")
                def load_weight_quarter(qi):
                    lo, hi = qi * (ND // 4), (qi + 1) * (ND // 4)
                    nc.sync.dma_start(
                        out=w_sb[:, lo * 512: hi * 512].rearrange(
                            "p (c m) -> p c m", c=hi - lo),
                        in_=wqkvT[lo * 128: hi * 128, :].rearrange(
                            "(c p) m -> p c m", p=128))

                # keep-warm matmuls: the PE would otherwise sit idle for
                # the initial weight/x DMA and pay the HAM half-clock ramp
                # when the first projections finally issue.
                for _ in range(_N_WARM):
                    wps = PSW.tile([128, 128], f32, tag="warm", name="warm")
                    nc.tensor.matmul(wps[:], lhsT=ident[:], rhs=ident[:],
                                     start=True, stop=True)

                # only the first quarter of the weights before the first x
                # tile; the rest interleave with tq5=0's x loads so the first
                # matmuls start earlier
                load_weight_quarter(0)

                for tq5 in range(NG):
                    pq = [PSA.tile([128, 512], f32, tag=f"pq{h}", name=f"pq{h}") for h in range(HPC)]
                    pk = PSA.tile([128, 512], f32, tag="pk")
                    pv = [PSA.tile([128, 128], f32, tag=f"pv{tb}", name=f"pv{tb}") for tb in range(4)]
                    for dcg in range(4):
                        # one 512KB DMA: 4 d-chunks x 512 tokens
                        xt = XP.tile([128, 4 * 512], bf16, tag="x")
                        nc.sync.dma_start(
                            out=xt[:].rearrange("p (c m) -> p c m", c=4),
                            in_=xT[dcg * 512:(dcg + 1) * 512,
                                   tq5 * 512:(tq5 + 1) * 512].rearrange(
                                       "(c p) m -> p c m", p=128))
                        if tq5 == 0 and dcg < 3:
                            load_weight_quarter(dcg + 1)
                        # tables + wo spread through tq5=1/2 x loads: the
                        # first rope (DVE) has ~100us of slack before its
                        # consumer, so keep the DMA queue clear for x tiles
                        if tq5 == 1 and dcg == 1:
                            nc.sync.dma_start(out=cos_sb[:], in_=cosT[:, :])
                        if tq5 == 1 and dcg == 3:
                            nc.sync.dma_start(out=nsin_sb[:], in_=nsinT[:, :])
                        if tq5 == 2 and dcg in (1, 3):
                            wh = dcg // 2
                            nc.sync.dma_start(
                                out=wo_sb[:, wh * D:(wh + 1) * D],
                                in_=woT[wh * 128:(wh + 1) * 128, :])
                        for dci in range(4):
                            dc = dcg * 4 + dci
                            xs = xt[:, dci * 512:(dci + 1) * 512]
                            st, sp = (dc == 0), (dc == ND - 1)
                            for h in range(HPC):
                                nc.tensor.matmul(
                                    pq[h][:],
                                    lhsT=w_sb[:, dc * 512 + h * 128: dc * 512 + (h + 1) * 128],
                                    rhs=xs, start=st, stop=sp)
                            nc.tensor.matmul(
                                pk[:], lhsT=w_sb[:, dc * 512 + 256: dc * 512 + 384],
                                rhs=xs, start=st, stop=sp)
                            # V in natural [token, dh] layout: x-chunk as lhsT
                            for tb in range(4):
                                nc.tensor.matmul(
                                    pv[tb][:],
                                    lhsT=xs[:, tb * 128:(tb + 1) * 128],
                                    rhs=w_sb[:, dc * 512 + 384: dc * 512 + 512],
                                    start=st, stop=sp)
                    # Vn copies first (Pool cannot read PSUM on real HW) so
                    # the pv banks drain promptly for the attention st tiles
                    for tb in range(4):
                        nc.vector.tensor_copy(
                            Vns[tq5][:, tb * 128:(tb + 1) * 128], pv[tb][:, :])
                    for h in range(HPC):
                        nc.vector.tensor_copy(QTs[h][tq5][:, :], pq[h][:, :])
                    nc.vector.tensor_copy(KTs[tq5][:, :], pk[:, :])
                    # RoPE is deferred one tq5 iteration: its cos/nsin
                    # tables are DMA'd during tq5=1, and the tile framework
                    # orders deps by emission, so rope(k) is emitted at the
                    # end of iteration k+1 (consumers are ~100us away).
                    # HW cannot partition-shift in a 2-input op, but a
                    # 1-input DVE copy can: multiply by nsin in place
                    # (rows 0:64 = +sin, 64:128 = -sin), then swap halves
                    # with two shifted copies to build rot*sin.
                    def emit_rope(idx):
                        tab = slice((idx * 512) % T, (idx * 512) % T + 512)
                        for tgt in [KTs[idx], QTs[0][idx], QTs[1][idx]]:
                            rtmp = RT2.tile([128, 512], bf16, tag="rtmp")
                            nc.vector.tensor_mul(rtmp[:], tgt[:, :],
                                                 nsin_sb[:, tab])
                            rtsw = RT2.tile([128, 512], bf16, tag="rtsw")
                            nc.vector.tensor_copy(rtsw[0:64, :], rtmp[64:128, :])
                            nc.vector.tensor_copy(rtsw[64:128, :], rtmp[0:64, :])
                            nc.vector.tensor_mul(tgt[:, :], tgt[:, :],
                                                 cos_sb[:, tab])
                            nc.gpsimd.tensor_add(tgt[:, :], tgt[:, :], rtsw[:])
                    if tq5 >= 1:
                        emit_rope(tq5 - 1)
                    if tq5 == NG - 1:
                        emit_rope(tq5)
            # ---------- attention + deferred output projection ----------
            # PSUM budget (16KB/partition): st 2x4KB + av 2KB + l/rbc 2KB
            # + wo 2x2KB = 16KB
            with tc.tile_pool(name="ptp", bufs=6) as PTP, \
                 tc.tile_pool(name="lap", bufs=3) as LAP, \
                 tc.tile_pool(name="rrp", bufs=2) as RRP, \
                 tc.tile_pool(name="osbp", bufs=4) as OSBP, \
                 tc.tile_pool(name="psAv", bufs=1, space="PSUM") as PSAV, \
                 tc.tile_pool(name="psL", bufs=1, space="PSUM") as PSL, \
                 tc.tile_pool(name="psC", bufs=2, space="PSUM") as PSC, \
                 tc.tile_pool(name="psSt", bufs=2, space="PSUM") as PSST:
                # prime PSUM slot allocation order so the st super-tiles
                # land on the banks that held pv0-3 (drained early on Pool)
                # rather than the pq/pk banks that drain last on DVE
                PSAV.tile([128, 512], f32, tag="av", name="prime_av")
                PSL.tile([128, 512], f32, tag="lr", name="prime_lr")
                PSC.tile([128, 512], f32, tag="wops", name="prime_wops")
                # also rotate the st pool: the first real st tile then takes
                # buf1 (fully free banks) while buf0 overlaps the still-
                # draining pv banks from the last projection block
                PSST.tile([128, 1024], f32, tag="st", name="prime_st")
                osb_cnt = [0]

                def wo_units(b, tsb, fine=False):
                    # output projection for one 512-token group as 8
                    # independently emittable units (one osb half each),
                    # 512-wide d-quarters with double-buffered PSUM.
                    # fine=True DMAs each quarter separately (shorter tail).
                    g = b * NSB + tsb
                    units = []
                    for tj in range(4):
                        for dhalf in range(2):
                            def unit(tj=tj, dhalf=dhalf):
                                tcx = (b * T + tsb * 512) // 128 + tj
                                osb = OSBP.tile([128, D // 2], bf16, tag="osb")
                                for dq in range(2):
                                    dcol = dhalf * 1024 + dq * 512
                                    wo_ps = PSC.tile([128, 512], f32, tag="wops")
                                    for h in range(HPC):
                                        nc.tensor.matmul(
                                            wo_ps[:],
                                            lhsT=AVTs[h][g][:, tj * 128:(tj + 1) * 128],
                                            rhs=wo_sb[:, h * D + dcol: h * D + dcol + 512],
                                            start=(h == 0), stop=(h == HPC - 1))
                                    # only DVE and Act can read PSUM; Act
                                    # also runs exp, so weight DVE 2:1
                                    eng = osb_cnt[0] % 3
                                    osb_cnt[0] += 1
                                    dst = osb[:, dq * 512:(dq + 1) * 512]
                                    if eng == 2:
                                        nc.scalar.copy(dst, wo_ps[:])
                                    else:
                                        nc.vector.tensor_copy(dst, wo_ps[:])
                                    if fine:
                                        nc.sync.dma_start(
                                            out=out[tcx * 128:(tcx + 1) * 128,
                                                    dcol:dcol + 512],
                                            in_=dst)
                                if not fine:
                                    nc.sync.dma_start(
                                        out=out[tcx * 128:(tcx + 1) * 128,
                                                dhalf * 1024:(dhalf + 1) * 1024],
                                        in_=osb[:])
                            units.append(unit)
                    return units

                # longest groups first: more full pairs early keeps the PE
                # ahead of the exp chain while there is no wo fill yet, and
                # the shortest group lands on the tail
                groups = [(0, 3), (0, 2), (0, 1), (0, 0),
                          (1, 3), (1, 2), (1, 1), (1, 0)]
                pending = []
                for b, tsb in groups:
                    n_sc = (tsb + 1) * 4
                    g = b * NSB + tsb
                    # interleave some of the previous group's wo units into
                    # this group's pair loop to fill exp-chain stalls; the
                    # rest stay for the group end to hide the l->recip->
                    # scale chain
                    drip_budget = [max(0, len(pending) - _WO_RESERVE)]
                    def drip():
                        if drip_budget[0] > 0 and pending:
                            pending.pop(0)()
                            drip_budget[0] -= 1
                    for h in range(HPC):
                        av_ps = PSAV.tile([128, 512], f32, tag="av")
                        pacc = LAP.tile([128, 512], fr, tag="pacc")

                        def emit_av(pt2, halves):
                            # the P-consuming half of a pair: causal-mask
                            # select, denominator accumulate, AV matmuls
                            for sc, c0, off in halves:
                                # zero the upper triangle of the diagonal
                                # 128-block post-exp on Pool (exp of
                                # unmasked scores is bounded, ~exp(1.5))
                                if sc >= tsb * 4:
                                    nc.gpsimd.affine_select(
                                        out=pt2[:, off + c0: off + c0 + 128],
                                        in_=pt2[:, off + c0: off + c0 + 128],
                                        compare_op=mybir.AluOpType.is_ge,
                                        fill=0.0,
                                        base=0,
                                        pattern=[[1, 128]],
                                        channel_multiplier=-1,
                                    )
                                # softmax denominators accumulate on Pool
                                # (saves one PE column-stream per chunk)
                                if sc == 0:
                                    nc.gpsimd.tensor_copy(pacc[:, :], pt2[:, 0:512])
                                else:
                                    nc.gpsimd.tensor_add(
                                        pacc[:, c0:512], pacc[:, c0:512],
                                        pt2[:, off + c0: off + 512])
                                nc.tensor.matmul(
                                    av_ps[:, c0:512],
                                    lhsT=Vns[b * NSB + sc // 4][:, (sc % 4) * 128:
                                                                (sc % 4 + 1) * 128],
                                    rhs=pt2[:, off + c0: off + 512],
                                    start=(sc == 0), stop=(sc == n_sc - 1))

                        # lag-2 software pipeline: pair i's AV work is
                        # emitted after pair i+1's S/exp, so on the in-order
                        # PE the AV matmuls never wait on their own exp
                        avq = []
                        for scp in range(n_sc // 2):
                            st2 = PSST.tile([128, 1024], f32, tag="st")
                            pt2 = PTP.tile([128, 1024], bf16, tag="pt")
                            halves = []
                            for k in (0, 1):
                                sc = 2 * scp + k
                                c0 = max(sc - tsb * 4, 0) * 128
                                off = k * 512
                                nc.tensor.matmul(
                                    st2[:, off + c0: off + 512],
                                    lhsT=KTs[b * NSB + sc // 4][:, (sc % 4) * 128:
                                                                (sc % 4 + 1) * 128],
                                    rhs=QTs[h][g][:, c0:512],
                                    start=True, stop=True)
                                halves.append((sc, c0, off))
                            if halves[1][1] == 0:
                                # both halves full-width: one paired exp
                                nc.scalar.activation(
                                    pt2[:, :], st2[:, :],
                                    mybir.ActivationFunctionType.Exp)
                            else:
                                for sc, c0, off in halves:
                                    nc.scalar.activation(
                                        pt2[:, off + c0: off + 512],
                                        st2[:, off + c0: off + 512],
                                        mybir.ActivationFunctionType.Exp)
                            avq.append((pt2, halves))
                            if len(avq) > 1:
                                emit_av(*avq.pop(0))
                            drip()
                        for item in avq:
                            emit_av(*item)
                        # copy first: frees av_ps for the next head's
                        # accumulation while the l chain runs
                        nc.vector.tensor_copy(AVTs[h][g][:, :], av_ps[:])
                        # all-ones lhsT: one matmul both sums pacc over s
                        # AND broadcasts l to every partition
                        l_bc = PSL.tile([128, 512], f32, tag="lr", name="l_bc")
                        nc.tensor.matmul(l_bc[:], lhsT=ones_sq[:], rhs=pacc[:],
                                         start=True, stop=True)
                        linv = RRP.tile([128, 512], bf16, tag="linv")
                        nc.vector.reciprocal(linv[:], l_bc[:])
                        nc.vector.tensor_mul(AVTs[h][g][:, :], AVTs[h][g][:, :],
                                             linv[:])
                    while pending:
                        pending.pop(0)()
                    pending = wo_units(b, tsb, fine=(b, tsb) == groups[-1])
                for u in pending:
                    u()

    if split_waits:
        _split_multi_waits(nc, mybir)
    return nc


def _host_inputs(x, wq, wk, wv, wo):
    import ml_dtypes
    bf = ml_dtypes.bfloat16
    xT = np.ascontiguousarray(x.reshape(BT, D).T.astype(bf))
    half = DH // 2
    inv = (1.0 / (ROPE_BASE ** (np.arange(half, dtype=np.float32) / half))).astype(np.float32)
    ang = np.arange(T, dtype=np.float32)[:, None] * inv[None, :]          # (T, 64)
    c = np.cos(ang).T.astype(np.float32)                                  # (64, T)
    s = np.sin(ang).T.astype(np.float32)
    cosT = np.ascontiguousarray(np.concatenate([c, c], axis=0).astype(bf))  # (128, T)
    nsinT = np.ascontiguousarray(np.concatenate([s, -s], axis=0).astype(bf))
    scale = np.float32(1.0 / np.sqrt(DH))
    in_maps = []
    for core in range(NCORES):
        kvh = core // 2
        wqkvT = np.concatenate([
            (wq[core * HPC * DH:(core + 1) * HPC * DH, :] * scale).T,
            wk[kvh * DH:(kvh + 1) * DH, :].T,
            wv[kvh * DH:(kvh + 1) * DH, :].T,
        ], axis=1)
        in_maps.append({
            "xT": xT,
            "wqkvT": np.ascontiguousarray(wqkvT.astype(bf)),
            "woT": np.ascontiguousarray(wo[:, core * HPC * DH:(core + 1) * HPC * DH].T.astype(bf)),
            "cosT": cosT,
            "nsinT": nsinT,
        })
    return in_maps


def kernel(x, wq, wk, wv, wo):
    _ensure_path()
    from concourse.bass_utils import run_bass_kernel_spmd

    x = np.asarray(x, dtype=np.float32)
    wq = np.asarray(wq, dtype=np.float32)
    wk = np.asarray(wk, dtype=np.float32)
    wv = np.asarray(wv, dtype=np.float32)
    wo = np.asarray(wo, dtype=np.float32)

    if "nc" not in _cache:
        _cache["nc"] = _build()
    nc = _cache["nc"]

    in_maps = _host_inputs(x, wq, wk, wv, wo)
    res = run_bass_kernel_spmd(nc, in_maps, list(range(NCORES)))
    acc = np.asarray(res.results[0]["out"]).astype(np.float32)
    for cidx in range(1, NCORES):
        acc = acc + np.asarray(res.results[cidx]["out"]).astype(np.float32)
    return acc.reshape(B, T, D)


# revision 51
# speedup vs baseline: 1.3596x; 1.0470x over previous
"""Tensor-parallel GQA multi-head attention for 8 Trainium2 NeuronCores.

Sharding: query heads (16) split 2-per-core; each core needs exactly one
KV head (GQA group); wq/wk/wv column-parallel, wo row-parallel; the
all-reduce after wo is done host-side (sum of 8 partial outputs).

Per-core dataflow (all matmul operands bf16, PSUM f32):
  QT/KT = Wqk^T-chunks (lhsT) x xT (rhs)       [dh, tokens]
  Vn    = xT-chunks (lhsT) x Wv^T (rhs)        [tokens, dh] natural layout
  rope  on QT/KT via partition-shifted DVE muls (no PE rotation matmul)
  S^T   = KT-chunk (lhsT) x QT (rhs)           [s, t]  (causal superblocks,
                                                paired into 1024-wide PSUM
                                                super-tiles for cheaper exp)
  P^T   = exp(S^T + causal mask)               (no max-subtraction: scores
                                                are bounded ~N(0, 1/9))
  l     = Pool-accumulated sum of P^T chunks, then one ones-matmul  [1, t]
  avT   = Vn-chunk (lhsT) x P^T (rhs)          [dh, t]; scaled by 1/l
  out   = avT-chunk (lhsT) x woT (rhs)         [t, d] partial, bf16 DMA out

Q/K/V and AV live in per-512-token-superblock tiles so the Tile
framework's whole-tile dependency tracking doesn't serialize attention
behind the last projection. Each group's output projection is deferred
by one group so the l->recip->broadcast->scale chain of the current
group hides under the previous group's wo matmuls.
"""

import numpy as np

B, T, D, H, KV = 2, 2048, 2048, 16, 4
DH = 128
NCORES = 8
HPC = H // NCORES          # 2 query heads per core
BT = B * T                 # 4096
ND = D // 128              # 16 contraction chunks
NSB = T // 512             # 4 causal superblocks per batch
NG = B * NSB               # 8 (batch, superblock) groups
ROPE_BASE = 10000.0
NEG = -1.0e4
_WO_RESERVE = 8            # wo units kept for the group-end flush
_N_WARM = 8                # keep-warm matmuls before the first projection

_cache = {}


def _ensure_path():
    try:
        import concourse.bass  # noqa: F401
    except ImportError:
        import sys
        for p in ("/opt/trn_rl_repo", "/root/.axon_site/_ro/trn_rl_repo"):
            if p not in sys.path:
                sys.path.insert(0, p)
        import concourse.bass  # noqa: F401


def _split_multi_waits(nc, mybir, max_waits=1):
    """This container's walrus rejects >1 sync-wait on one instruction
    (seen on the Tile tail drain). Move extra waits onto preceding NoOps
    on the same engine; per-engine program order preserves semantics."""
    for bb in nc.main_func.blocks:
        new_insts = []
        for ins in bb.instructions:
            si = getattr(ins, "sync_info", None)
            if si is not None and si.on_wait and len(si.on_wait) > max_waits:
                waits = list(si.on_wait)
                extra, keep = waits[:-max_waits], waits[-max_waits:]
                for w in extra:
                    new_insts.append(
                        mybir.InstNoOp(
                            name=nc.get_next_instruction_name(),
                            sync_info=mybir.SyncInfo(on_wait=[w], on_update=[]),
                            bass_nofuse=True,
                            engine=ins.engine,
                            ins=[],
                            outs=[],
                        )
                    )
                si.on_wait = keep
            new_insts.append(ins)
        bb.instructions = new_insts


def _build(split_waits=True):
    _ensure_path()
    import concourse.bass as bass
    import concourse.mybir as mybir
    import concourse.tile as tile
    from concourse.masks import make_identity

    f32 = mybir.dt.float32
    fr = mybir.dt.float32r
    bf16 = mybir.dt.bfloat16
    nc = bass.Bass()

    xT = nc.declare_dram_parameter("xT", [D, BT], bf16, isOutput=False)
    # per 128-row chunk c: cols 0:128 q head0, 128:256 q head1, 256:384 k,
    # 384:512 v
    wqkvT = nc.declare_dram_parameter("wqkvT", [D, 4 * DH], bf16, isOutput=False)
    woT = nc.declare_dram_parameter("woT", [HPC * DH, D], bf16, isOutput=False)
    cosT = nc.declare_dram_parameter("cosT", [DH, T], bf16, isOutput=False)
    # rows 0:64 hold +sin, rows 64:128 hold -sin
    nsinT = nc.declare_dram_parameter("nsinT", [DH, T], bf16, isOutput=False)
    out = nc.declare_dram_parameter("out", [BT, D], bf16, isOutput=True)

    with nc.allow_low_precision(reason="bf16 fast matmul path"), \
         tile.TileContext(nc) as tc:
        with tc.tile_pool(name="persist", bufs=1) as P:
            ident = P.tile([128, 128], f32, tag="ident")
            ones_sq = P.tile([128, 128], fr, tag="ones_sq")
            ones_sqf = P.tile([128, 128], f32, tag="ones_sqf")
            ones_rf = P.tile([1, 128], f32, tag="ones_rf")
            make_identity(nc, ident[:])
            nc.gpsimd.memset(ones_sqf[:], 1.0)
            nc.vector.tensor_copy(ones_sq[:], ones_sqf[:])
            nc.gpsimd.memset(ones_rf[:], 1.0)
            # dummy exp: pull the 1.3us activation-table load into phase A
            # instead of paying it on the first real softmax exp
            dummy_e = P.tile([1, 1], f32, tag="dummy_e")
            nc.scalar.activation(dummy_e[:], ones_rf[0:1, 0:1],
                                 mybir.ActivationFunctionType.Exp)

            cos_sb = P.tile([128, T], bf16, tag="cos")
            nsin_sb = P.tile([128, T], bf16, tag="nsin")
            wo_sb = P.tile([128, HPC * D], bf16, tag="wo")
            # per-superblock tiles: fine-grained deps let attention start
            # on a superblock as soon as its projections+rope finish
            QTs = [[P.tile([128, 512], bf16, tag=f"qt{h}_{g}", name=f"qt{h}_{g}")
                    for g in range(NG)] for h in range(HPC)]
            KTs = [P.tile([128, 512], bf16, tag=f"kt{g}", name=f"kt{g}")
                   for g in range(NG)]
            Vns = [P.tile([128, 512], bf16, tag=f"vn{g}", name=f"vn{g}")
                   for g in range(NG)]
            AVTs = [[P.tile([128, 512], bf16, tag=f"avt{h}_{g}", name=f"avt{h}_{g}")
                     for g in range(NG)] for h in range(HPC)]

            # pools shared by both regions. PSUM budget is exact 16KB
            # per partition in each region:
            #   region A/2: psA 8KB + psSt2 4KB + psAv 2KB + psL 2KB
            #   region 3:   psSt3 8KB + psC 4KB + psAv 2KB + psL 2KB
            with tc.tile_pool(name="ptp", bufs=6) as PTP, \
                 tc.tile_pool(name="lap", bufs=3) as LAP, \
                 tc.tile_pool(name="rrp", bufs=2) as RRP, \
                 tc.tile_pool(name="osbp", bufs=4) as OSBP, \
                 tc.tile_pool(name="psL", bufs=1, space="PSUM") as PSL:
              # ------- phase A: QKV projections + RoPE, with the b=0
              # attention groups micro-interleaved into tq5>=4 so the
              # projection stream hides their exp-chain latency -------
              with tc.tile_pool(name="wpool", bufs=1) as WP, \
                 tc.tile_pool(name="xp", bufs=6) as XP, \
                 tc.tile_pool(name="ropetA", bufs=4) as RT2, \
                 tc.tile_pool(name="psA", bufs=1, space="PSUM") as PSA, \
                 tc.tile_pool(name="psSt2", bufs=3, space="PSUM") as PSST2:
                w_sb = WP.tile([128, ND * 4 * DH], bf16, tag="w")
                def load_weight_quarter(qi):
                    lo, hi = qi * (ND // 4), (qi + 1) * (ND // 4)
                    nc.sync.dma_start(
                        out=w_sb[:, lo * 512: hi * 512].rearrange(
                            "p (c m) -> p c m", c=hi - lo),
                        in_=wqkvT[lo * 128: hi * 128, :].rearrange(
                            "(c p) m -> p c m", p=128))

                # keep-warm matmuls: the PE would otherwise sit idle for
                # the initial weight/x DMA and pay the HAM half-clock ramp
                # when the first projections finally issue.
                for _ in range(_N_WARM):
                    wps = PSA.tile([128, 512], f32, tag="pv", name="warm")
                    nc.tensor.matmul(wps[:, 0:128], lhsT=ident[:], rhs=ident[:],
                                     start=True, stop=True)

                # only the first quarter of the weights before the first x
                # tile; the rest interleave with tq5=0's x loads so the first
                # matmuls start earlier
                load_weight_quarter(0)

                # --- b=0 attention as a queue of micro-ops (one S+exp or
                # one mask+denominator+AV or one normalization chain each),
                # popped between projection chunks of tq5>=4. Items are
                # ordered with the AV half one slot behind its S half, so
                # the in-order PE never waits on an exp. ---
                b0q = []
                b0_state = {}

                def b0_attention_items():
                    items = []
                    for tsb in (3, 2, 1, 0):
                        g = tsb
                        n_sc = (tsb + 1) * 4
                        for h in range(HPC):
                            key = (g, h)
                            pend = []

                            def s_part(sc, g=g, h=h, tsb=tsb, key=key):
                                c0 = max(sc - tsb * 4, 0) * 128
                                if sc == 0:
                                    b0_state[key] = (
                                        PSL.tile([128, 512], f32, tag="lr",
                                                 name=f"b0av{g}_{h}"),
                                        LAP.tile([128, 512], fr, tag="pacc",
                                                 name=f"b0pacc{g}_{h}"))
                                st1 = PSST2.tile([128, 512], f32, tag="st2",
                                                 name=f"b0st{g}_{h}_{sc}")
                                pt1 = PTP.tile([128, 512], bf16, tag="pt2s",
                                               name=f"b0pt{g}_{h}_{sc}")
                                nc.tensor.matmul(
                                    st1[:, c0:512],
                                    lhsT=KTs[sc // 4][:, (sc % 4) * 128:
                                                      (sc % 4 + 1) * 128],
                                    rhs=QTs[h][g][:, c0:512],
                                    start=True, stop=True)
                                nc.scalar.activation(
                                    pt1[:, c0:512], st1[:, c0:512],
                                    mybir.ActivationFunctionType.Exp)
                                return pt1

                            def av_part(sc, pt1, g=g, h=h, tsb=tsb, key=key,
                                        n_sc=n_sc):
                                c0 = max(sc - tsb * 4, 0) * 128
                                av_ps, pacc = b0_state[key]
                                if sc >= tsb * 4:
                                    nc.gpsimd.affine_select(
                                        out=pt1[:, c0:c0 + 128],
                                        in_=pt1[:, c0:c0 + 128],
                                        compare_op=mybir.AluOpType.is_ge,
                                        fill=0.0,
                                        base=0,
                                        pattern=[[1, 128]],
                                        channel_multiplier=-1,
                                    )
                                if sc == 0:
                                    nc.gpsimd.tensor_copy(pacc[:, :], pt1[:, :])
                                else:
                                    nc.gpsimd.tensor_add(
                                        pacc[:, c0:512], pacc[:, c0:512],
                                        pt1[:, c0:512])
                                nc.tensor.matmul(
                                    av_ps[:, c0:512],
                                    lhsT=Vns[sc // 4][:, (sc % 4) * 128:
                                                      (sc % 4 + 1) * 128],
                                    rhs=pt1[:, c0:512],
                                    start=(sc == 0), stop=(sc == n_sc - 1))

                            def mk_s(sc, s_part=s_part, pend=pend):
                                def run(sc=sc, s_part=s_part, pend=pend):
                                    pt1 = s_part(sc)
                                    pend.append((sc, pt1))
                                return run

                            def mk_av(av_part=av_part, pend=pend):
                                def run(av_part=av_part, pend=pend):
                                    sc, pt1 = pend.pop(0)
                                    av_part(sc, pt1)
                                return run

                            def chain(g=g, h=h, key=key):
                                av_ps, pacc = b0_state[key]
                                nc.vector.tensor_copy(AVTs[h][g][:, :], av_ps[:])
                                l_bc = PSL.tile([128, 512], f32, tag="lr",
                                                name=f"b0l{g}_{h}")
                                nc.tensor.matmul(l_bc[:], lhsT=ones_sq[:],
                                                 rhs=pacc[:], start=True,
                                                 stop=True)
                                linv = RRP.tile([128, 512], bf16, tag="linv")
                                nc.vector.reciprocal(linv[:], l_bc[:])
                                nc.vector.tensor_mul(AVTs[h][g][:, :],
                                                     AVTs[h][g][:, :], linv[:])

                            items.append(mk_s(0))
                            for sc in range(1, n_sc):
                                items.append(mk_s(sc))
                                items.append(mk_av())
                            items.append(mk_av())
                            items.append(chain)
                    return items

                b0_points = [4 * 16]  # dci points in tq5 4..7
                def b0_drip():
                    if not b0q:
                        return
                    pts = b0_points[0]
                    k = -(-len(b0q) // max(pts, 1))
                    for _ in range(min(k, len(b0q))):
                        b0q.pop(0)()
                    b0_points[0] = max(pts - 1, 1)

                for tq5 in range(NG):
                    pq = [PSA.tile([128, 512], f32, tag=f"pq{h}", name=f"pq{h}") for h in range(HPC)]
                    pk = PSA.tile([128, 512], f32, tag="pk")
                    # all four V accumulators share one PSUM bank (PSUM
                    # slots are bank-granular; separate accumulation chains
                    # in disjoint column ranges of a bank are fine)
                    pv = PSA.tile([128, 512], f32, tag="pv")
                    xts = []
                    for dcg in range(4):
                        # one 512KB DMA: 4 d-chunks x 512 tokens
                        xt = XP.tile([128, 4 * 512], bf16, tag="x")
                        nc.sync.dma_start(
                            out=xt[:].rearrange("p (c m) -> p c m", c=4),
                            in_=xT[dcg * 512:(dcg + 1) * 512,
                                   tq5 * 512:(tq5 + 1) * 512].rearrange(
                                       "(c p) m -> p c m", p=128))
                        if tq5 == 0 and dcg < 3:
                            load_weight_quarter(dcg + 1)
                        # tables + wo spread through tq5=1/2 x loads: the
                        # first rope (DVE) has ~100us of slack before its
                        # consumer, so keep the DMA queue clear for x tiles
                        if tq5 == 1 and dcg == 1:
                            nc.sync.dma_start(out=cos_sb[:], in_=cosT[:, :])
                        if tq5 == 1 and dcg == 3:
                            nc.sync.dma_start(out=nsin_sb[:], in_=nsinT[:, :])
                        if tq5 == 2 and dcg in (1, 3):
                            wh = dcg // 2
                            nc.sync.dma_start(
                                out=wo_sb[:, wh * D:(wh + 1) * D],
                                in_=woT[wh * 128:(wh + 1) * 128, :])
                        xts.append(xt)
                        for dci in range(4):
                            dc = dcg * 4 + dci
                            xs = xt[:, dci * 512:(dci + 1) * 512]
                            st, sp = (dc == 0), (dc == ND - 1)
                            for h in range(HPC):
                                nc.tensor.matmul(
                                    pq[h][:],
                                    lhsT=w_sb[:, dc * 512 + h * 128: dc * 512 + (h + 1) * 128],
                                    rhs=xs, start=st, stop=sp)
                            nc.tensor.matmul(
                                pk[:], lhsT=w_sb[:, dc * 512 + 256: dc * 512 + 384],
                                rhs=xs, start=st, stop=sp)
                            b0_drip()
                    # V in natural [token, dh] layout, x-chunks as lhsT.
                    # One tb's 16-chunk chain finishes before the next
                    # starts: the four accumulators share one PSUM bank and
                    # a bank allows only one open accumulation group.
                    for tb in range(4):
                        for dc in range(ND):
                            xs = xts[dc // 4][:, (dc % 4) * 512:
                                              (dc % 4 + 1) * 512]
                            nc.tensor.matmul(
                                pv[:, tb * 128:(tb + 1) * 128],
                                lhsT=xs[:, tb * 128:(tb + 1) * 128],
                                rhs=w_sb[:, dc * 512 + 384: dc * 512 + 512],
                                start=(dc == 0), stop=(dc == ND - 1))
                        b0_drip()
                    # Vn copy first so the pv bank drains promptly
                    nc.vector.tensor_copy(Vns[tq5][:, :], pv[:, :])
                    for h in range(HPC):
                        nc.vector.tensor_copy(QTs[h][tq5][:, :], pq[h][:, :])
                    nc.vector.tensor_copy(KTs[tq5][:, :], pk[:, :])
                    # RoPE is deferred one tq5 iteration: its cos/nsin
                    # tables are DMA'd during tq5=1, and the tile framework
                    # orders deps by emission, so rope(k) is emitted at the
                    # end of iteration k+1 (consumers are ~100us away).
                    # HW cannot partition-shift in a 2-input op, but a
                    # 1-input DVE copy can: multiply by nsin in place
                    # (rows 0:64 = +sin, 64:128 = -sin), then swap halves
                    # with two shifted copies to build rot*sin.
                    def emit_rope(idx):
                        tab = slice((idx * 512) % T, (idx * 512) % T + 512)
                        for tgt in [KTs[idx], QTs[0][idx], QTs[1][idx]]:
                            rtmp = RT2.tile([128, 512], bf16, tag="rtmp")
                            nc.vector.tensor_mul(rtmp[:], tgt[:, :],
                                                 nsin_sb[:, tab])
                            rtsw = RT2.tile([128, 512], bf16, tag="rtsw")
                            nc.vector.tensor_copy(rtsw[0:64, :], rtmp[64:128, :])
                            nc.vector.tensor_copy(rtsw[64:128, :], rtmp[0:64, :])
                            nc.vector.tensor_mul(tgt[:, :], tgt[:, :],
                                                 cos_sb[:, tab])
                            nc.gpsimd.tensor_add(tgt[:, :], tgt[:, :], rtsw[:])
                    if 1 <= tq5 <= 2:
                        emit_rope(tq5 - 1)
                    if tq5 == 3:
                        emit_rope(2)
                        emit_rope(3)
                    if 5 <= tq5 <= 6:
                        emit_rope(tq5 - 1)
                    if tq5 == NG - 1:
                        emit_rope(tq5 - 1)
                        emit_rope(tq5)
                    if tq5 == 3:
                        b0q.extend(b0_attention_items())
                    if tq5 == NG - 1:
                        while b0q:
                            b0q.pop(0)()
              # ------- region 3: b=1 attention (paired exp) + ALL
              # groups' output projections -------
              with tc.tile_pool(name="psC", bufs=3, space="PSUM") as PSC, \
                 tc.tile_pool(name="psSt", bufs=2, space="PSUM") as PSST:
                osb_cnt = [0]

                def wo_units(b, tsb, fine=False):
                    # output projection for one 512-token group as 8
                    # independently emittable units (one osb half each),
                    # 512-wide d-quarters with double-buffered PSUM.
                    # fine=True DMAs each quarter separately (shorter tail).
                    g = b * NSB + tsb
                    units = []
                    for tj in range(4):
                        for dhalf in range(2):
                            def unit(tj=tj, dhalf=dhalf):
                                tcx = (b * T + tsb * 512) // 128 + tj
                                osb = OSBP.tile([128, D // 2], bf16, tag="osb")
                                for dq in range(2):
                                    dcol = dhalf * 1024 + dq * 512
                                    wo_ps = PSC.tile([128, 512], f32, tag="wops")
                                    for h in range(HPC):
                                        nc.tensor.matmul(
                                            wo_ps[:],
                                            lhsT=AVTs[h][g][:, tj * 128:(tj + 1) * 128],
                                            rhs=wo_sb[:, h * D + dcol: h * D + dcol + 512],
                                            start=(h == 0), stop=(h == HPC - 1))
                                    # only DVE and Act can read PSUM; Act
                                    # also runs exp, so weight DVE 2:1
                                    eng = osb_cnt[0] % 3
                                    osb_cnt[0] += 1
                                    dst = osb[:, dq * 512:(dq + 1) * 512]
                                    if eng == 2:
                                        nc.scalar.copy(dst, wo_ps[:])
                                    else:
                                        nc.vector.tensor_copy(dst, wo_ps[:])
                                    if fine:
                                        nc.sync.dma_start(
                                            out=out[tcx * 128:(tcx + 1) * 128,
                                                    dcol:dcol + 512],
                                            in_=dst)
                                if not fine:
                                    nc.sync.dma_start(
                                        out=out[tcx * 128:(tcx + 1) * 128,
                                                dhalf * 1024:(dhalf + 1) * 1024],
                                        in_=osb[:])
                            units.append(unit)
                    return units

                # b=0 attention already ran inside phase A; here the b=1
                # groups run with the b=0 groups' wo blocks as stall fill,
                # two wo groups flushed after each b=1 group's heads
                groups = [(1, 3), (1, 2), (1, 1), (1, 0)]
                flush_after = {(1, 3): [(0, 3), (0, 2)],
                               (1, 2): [(0, 1), (0, 0)],
                               (1, 1): [(1, 3)],
                               (1, 0): [(1, 2), (1, 1)]}
                def drip():
                    pass
                for b, tsb in groups:
                    n_sc = (tsb + 1) * 4
                    g = b * NSB + tsb
                    for h in range(HPC):
                        av_ps = PSL.tile([128, 512], f32, tag="lr",
                                         name="av_ps")
                        pacc = LAP.tile([128, 512], fr, tag="pacc")

                        def emit_av(pt2, halves):
                            # the P-consuming half of a pair: causal-mask
                            # select, denominator accumulate, AV matmuls
                            for sc, c0, off in halves:
                                # zero the upper triangle of the diagonal
                                # 128-block post-exp on Pool (exp of
                                # unmasked scores is bounded, ~exp(1.5))
                                if sc >= tsb * 4:
                                    nc.gpsimd.affine_select(
                                        out=pt2[:, off + c0: off + c0 + 128],
                                        in_=pt2[:, off + c0: off + c0 + 128],
                                        compare_op=mybir.AluOpType.is_ge,
                                        fill=0.0,
                                        base=0,
                                        pattern=[[1, 128]],
                                        channel_multiplier=-1,
                                    )
                                # softmax denominators accumulate on Pool
                                # (saves one PE column-stream per chunk)
                                if sc == 0:
                                    nc.gpsimd.tensor_copy(pacc[:, :], pt2[:, 0:512])
                                else:
                                    nc.gpsimd.tensor_add(
                                        pacc[:, c0:512], pacc[:, c0:512],
                                        pt2[:, off + c0: off + 512])
                                nc.tensor.matmul(
                                    av_ps[:, c0:512],
                                    lhsT=Vns[b * NSB + sc // 4][:, (sc % 4) * 128:
                                                                (sc % 4 + 1) * 128],
                                    rhs=pt2[:, off + c0: off + 512],
                                    start=(sc == 0), stop=(sc == n_sc - 1))

                        # lag-2 software pipeline: pair i's AV work is
                        # emitted after pair i+1's S/exp, so on the in-order
                        # PE the AV matmuls never wait on their own exp
                        avq = []
                        for scp in range(n_sc // 2):
                            st2 = PSST.tile([128, 1024], f32, tag="st")
                            pt2 = PTP.tile([128, 1024], bf16, tag="pt")
                            halves = []
                            for k in (0, 1):
                                sc = 2 * scp + k
                                c0 = max(sc - tsb * 4, 0) * 128
                                off = k * 512
                                nc.tensor.matmul(
                                    st2[:, off + c0: off + 512],
                                    lhsT=KTs[b * NSB + sc // 4][:, (sc % 4) * 128:
                                                                (sc % 4 + 1) * 128],
                                    rhs=QTs[h][g][:, c0:512],
                                    start=True, stop=True)
                                halves.append((sc, c0, off))
                            if halves[1][1] == 0:
                                # both halves full-width: one paired exp
                                nc.scalar.activation(
                                    pt2[:, :], st2[:, :],
                                    mybir.ActivationFunctionType.Exp)
                            else:
                                for sc, c0, off in halves:
                                    nc.scalar.activation(
                                        pt2[:, off + c0: off + 512],
                                        st2[:, off + c0: off + 512],
                                        mybir.ActivationFunctionType.Exp)
                            avq.append((pt2, halves))
                            if len(avq) > 1:
                                emit_av(*avq.pop(0))
                            drip()
                        for item in avq:
                            emit_av(*item)
                        # copy first: frees av_ps for the next head's
                        # accumulation while the l chain runs
                        nc.vector.tensor_copy(AVTs[h][g][:, :], av_ps[:])
                        # all-ones lhsT: one matmul both sums pacc over s
                        # AND broadcasts l to every partition
                        l_bc = PSL.tile([128, 512], f32, tag="lr", name="l_bc")
                        nc.tensor.matmul(l_bc[:], lhsT=ones_sq[:], rhs=pacc[:],
                                         start=True, stop=True)
                        linv = RRP.tile([128, 512], bf16, tag="linv")
                        nc.vector.reciprocal(linv[:], l_bc[:])
                        nc.vector.tensor_mul(AVTs[h][g][:, :], AVTs[h][g][:, :],
                                             linv[:])
                    for wg in flush_after[(b, tsb)]:
                        for u in wo_units(*wg):
                            u()
                for u in wo_units(1, 0, fine=True):
                    u()

    if split_waits:
        _split_multi_waits(nc, mybir)
    return nc


def _host_inputs(x, wq, wk, wv, wo):
    import ml_dtypes
    bf = ml_dtypes.bfloat16
    xT = np.ascontiguousarray(x.reshape(BT, D).T.astype(bf))
    half = DH // 2
    inv = (1.0 / (ROPE_BASE ** (np.arange(half, dtype=np.float32) / half))).astype(np.float32)
    ang = np.arange(T, dtype=np.float32)[:, None] * inv[None, :]          # (T, 64)
    c = np.cos(ang).T.astype(np.float32)                                  # (64, T)
    s = np.sin(ang).T.astype(np.float32)
    cosT = np.ascontiguousarray(np.concatenate([c, c], axis=0).astype(bf))  # (128, T)
    nsinT = np.ascontiguousarray(np.concatenate([s, -s], axis=0).astype(bf))
    scale = np.float32(1.0 / np.sqrt(DH))
    in_maps = []
    for core in range(NCORES):
        kvh = core // 2
        wqkvT = np.concatenate([
            (wq[core * HPC * DH:(core + 1) * HPC * DH, :] * scale).T,
            wk[kvh * DH:(kvh + 1) * DH, :].T,
            wv[kvh * DH:(kvh + 1) * DH, :].T,
        ], axis=1)
        in_maps.append({
            "xT": xT,
            "wqkvT": np.ascontiguousarray(wqkvT.astype(bf)),
            "woT": np.ascontiguousarray(wo[:, core * HPC * DH:(core + 1) * HPC * DH].T.astype(bf)),
            "cosT": cosT,
            "nsinT": nsinT,
        })
    return in_maps


def kernel(x, wq, wk, wv, wo):
    _ensure_path()
    from concourse.bass_utils import run_bass_kernel_spmd

    x = np.asarray(x, dtype=np.float32)
    wq = np.asarray(wq, dtype=np.float32)
    wk = np.asarray(wk, dtype=np.float32)
    wv = np.asarray(wv, dtype=np.float32)
    wo = np.asarray(wo, dtype=np.float32)

    if "nc" not in _cache:
        _cache["nc"] = _build()
    nc = _cache["nc"]

    in_maps = _host_inputs(x, wq, wk, wv, wo)
    res = run_bass_kernel_spmd(nc, in_maps, list(range(NCORES)))
    acc = np.asarray(res.results[0]["out"]).astype(np.float32)
    for cidx in range(1, NCORES):
        acc = acc + np.asarray(res.results[cidx]["out"]).astype(np.float32)
    return acc.reshape(B, T, D)


# revision 58
# speedup vs baseline: 1.3642x; 1.0033x over previous
"""Tensor-parallel GQA multi-head attention for 8 Trainium2 NeuronCores.

Sharding: query heads (16) split 2-per-core; each core needs exactly one
KV head (GQA group); wq/wk/wv column-parallel, wo row-parallel; the
all-reduce after wo is done host-side (sum of 8 partial outputs).

Per-core dataflow (all matmul operands bf16, PSUM f32):
  QT/KT = Wqk^T-chunks (lhsT) x xT (rhs)       [dh, tokens]
  Vn    = xT-chunks (lhsT) x Wv^T (rhs)        [tokens, dh] natural layout
  rope  on QT/KT on DVE: mul by [sin;-sin], swap halves with two
          partition-shifted copies (1-input copies may shift partitions
          on HW; 2-input ops may not), mul by cos, add on Pool
  S^T   = KT-chunk (lhsT) x QT (rhs)           [s, t]  (causal superblocks)
  P^T   = exp(S^T)                             (no max-subtraction: scores
                                                are bounded ~N(0, 1/9));
          causal mask = Pool affine_select zeroing the diag upper triangle
  l     = Pool-accumulated sum of P^T chunks, then one all-ones matmul
          that both reduces and broadcasts [128, t]; 1/l applied on DVE
  avT   = Vn-chunk (lhsT) x P^T (rhs)          [dh, t]; scaled by 1/l
  out   = avT-chunk (lhsT) x woT (rhs)         [t, d] partial, bf16 DMA out

Schedule: three regions, PSUM exactly 8 banks each.
 1. tq5=0..3 projections (Q/K per d-chunk; V as four sequential
    per-token-block chains sharing one PSUM bank).
 2. tq5=4..7 projections with the b=0 attention groups micro-interleaved
    between projection chunks (one S+exp, one mask+denom+AV, or one
    normalization chain per slot) so the in-order PE never stalls on the
    softmax chain; av/l_bc share one PSUM bank (their chains alternate).
 3. b=1 attention with paired 1024-wide exp super-tiles, plus all eight
    groups' output projections as fill between/after them, wo quarters
    triple-buffered in PSUM.
Q/K/V and AV live in per-512-token-superblock tiles so whole-tile
dependency tracking never serializes attention behind later projections.
"""

import numpy as np

B, T, D, H, KV = 2, 2048, 2048, 16, 4
DH = 128
NCORES = 8
HPC = H // NCORES          # 2 query heads per core
BT = B * T                 # 4096
ND = D // 128              # 16 contraction chunks
NSB = T // 512             # 4 causal superblocks per batch
NG = B * NSB               # 8 (batch, superblock) groups
ROPE_BASE = 10000.0
NEG = -1.0e4
_WO_RESERVE = 8            # wo units kept for the group-end flush
_N_WARM = 4                # keep-warm matmuls before the first projection

_cache = {}


def _ensure_path():
    try:
        import concourse.bass  # noqa: F401
    except ImportError:
        import sys
        for p in ("/opt/trn_rl_repo", "/root/.axon_site/_ro/trn_rl_repo"):
            if p not in sys.path:
                sys.path.insert(0, p)
        import concourse.bass  # noqa: F401


def _split_multi_waits(nc, mybir, max_waits=1):
    """This container's walrus rejects >1 sync-wait on one instruction
    (seen on the Tile tail drain). Move extra waits onto preceding NoOps
    on the same engine; per-engine program order preserves semantics."""
    for bb in nc.main_func.blocks:
        new_insts = []
        for ins in bb.instructions:
            si = getattr(ins, "sync_info", None)
            if si is not None and si.on_wait and len(si.on_wait) > max_waits:
                waits = list(si.on_wait)
                extra, keep = waits[:-max_waits], waits[-max_waits:]
                for w in extra:
                    new_insts.append(
                        mybir.InstNoOp(
                            name=nc.get_next_instruction_name(),
                            sync_info=mybir.SyncInfo(on_wait=[w], on_update=[]),
                            bass_nofuse=True,
                            engine=ins.engine,
                            ins=[],
                            outs=[],
                        )
                    )
                si.on_wait = keep
            new_insts.append(ins)
        bb.instructions = new_insts


def _build(split_waits=True):
    _ensure_path()
    import concourse.bass as bass
    import concourse.mybir as mybir
    import concourse.tile as tile
    from concourse.masks import make_identity

    f32 = mybir.dt.float32
    fr = mybir.dt.float32r
    bf16 = mybir.dt.bfloat16
    nc = bass.Bass()

    xT = nc.declare_dram_parameter("xT", [D, BT], bf16, isOutput=False)
    # per 128-row chunk c: cols 0:128 q head0, 128:256 q head1, 256:384 k,
    # 384:512 v
    wqkvT = nc.declare_dram_parameter("wqkvT", [D, 4 * DH], bf16, isOutput=False)
    woT = nc.declare_dram_parameter("woT", [HPC * DH, D], bf16, isOutput=False)
    cosT = nc.declare_dram_parameter("cosT", [DH, T], bf16, isOutput=False)
    # rows 0:64 hold +sin, rows 64:128 hold -sin
    nsinT = nc.declare_dram_parameter("nsinT", [DH, T], bf16, isOutput=False)
    out = nc.declare_dram_parameter("out", [BT, D], bf16, isOutput=True)

    with nc.allow_low_precision(reason="bf16 fast matmul path"), \
         tile.TileContext(nc) as tc:
        with tc.tile_pool(name="persist", bufs=1) as P:
            ident = P.tile([128, 128], f32, tag="ident")
            ones_sq = P.tile([128, 128], fr, tag="ones_sq")
            ones_sqf = P.tile([128, 128], f32, tag="ones_sqf")
            ones_rf = P.tile([1, 128], f32, tag="ones_rf")
            make_identity(nc, ident[:])
            nc.gpsimd.memset(ones_sqf[:], 1.0)
            nc.vector.tensor_copy(ones_sq[:], ones_sqf[:])
            nc.gpsimd.memset(ones_rf[:], 1.0)
            # dummy exp: pull the 1.3us activation-table load into phase A
            # instead of paying it on the first real softmax exp
            dummy_e = P.tile([1, 1], f32, tag="dummy_e")
            nc.scalar.activation(dummy_e[:], ones_rf[0:1, 0:1],
                                 mybir.ActivationFunctionType.Exp)

            cos_sb = P.tile([128, T], bf16, tag="cos")
            nsin_sb = P.tile([128, T], bf16, tag="nsin")
            wo_sb = P.tile([128, HPC * D], bf16, tag="wo")
            # per-superblock tiles: fine-grained deps let attention start
            # on a superblock as soon as its projections+rope finish
            QTs = [[P.tile([128, 512], bf16, tag=f"qt{h}_{g}", name=f"qt{h}_{g}")
                    for g in range(NG)] for h in range(HPC)]
            KTs = [P.tile([128, 512], bf16, tag=f"kt{g}", name=f"kt{g}")
                   for g in range(NG)]
            Vns = [P.tile([128, 512], bf16, tag=f"vn{g}", name=f"vn{g}")
                   for g in range(NG)]
            AVTs = [[P.tile([128, 512], bf16, tag=f"avt{h}_{g}", name=f"avt{h}_{g}")
                     for g in range(NG)] for h in range(HPC)]

            # pools shared by both regions. PSUM budget is exact 16KB
            # per partition in each region:
            #   region A/2: psA 8KB + psSt2 4KB + psAv 2KB + psL 2KB
            #   region 3:   psSt3 8KB + psC 4KB + psAv 2KB + psL 2KB
            with tc.tile_pool(name="ptp", bufs=6) as PTP, \
                 tc.tile_pool(name="lap", bufs=3) as LAP, \
                 tc.tile_pool(name="rrp", bufs=2) as RRP, \
                 tc.tile_pool(name="osbp", bufs=4) as OSBP, \
                 tc.tile_pool(name="psL", bufs=1, space="PSUM") as PSL:
              # ------- phase A: QKV projections + RoPE, with the b=0
              # attention groups micro-interleaved into tq5>=4 so the
              # projection stream hides their exp-chain latency -------
              with tc.tile_pool(name="wpool", bufs=1) as WP, \
                 tc.tile_pool(name="xp", bufs=6) as XP, \
                 tc.tile_pool(name="ropetA", bufs=4) as RT2, \
                 tc.tile_pool(name="psA", bufs=1, space="PSUM") as PSA, \
                 tc.tile_pool(name="psSt2", bufs=3, space="PSUM") as PSST2:
                w_sb = WP.tile([128, ND * 4 * DH], bf16, tag="w")
                def load_weight_quarter(qi):
                    lo, hi = qi * (ND // 4), (qi + 1) * (ND // 4)
                    nc.sync.dma_start(
                        out=w_sb[:, lo * 512: hi * 512].rearrange(
                            "p (c m) -> p c m", c=hi - lo),
                        in_=wqkvT[lo * 128: hi * 128, :].rearrange(
                            "(c p) m -> p c m", p=128))

                # keep-warm matmuls: the PE would otherwise sit idle for
                # the initial weight/x DMA and pay the HAM half-clock ramp
                # when the first projections finally issue.
                for _ in range(_N_WARM):
                    wps = PSA.tile([128, 512], f32, tag="pv", name="warm")
                    nc.tensor.matmul(wps[:, 0:128], lhsT=ident[:], rhs=ident[:],
                                     start=True, stop=True)

                # only the first quarter of the weights before the first x
                # tile; the rest interleave with tq5=0's x loads so the first
                # matmuls start earlier
                load_weight_quarter(0)

                # --- b=0 attention as a queue of micro-ops (one S+exp or
                # one mask+denominator+AV or one normalization chain each),
                # popped between projection chunks of tq5>=4. Items are
                # ordered with the AV half one slot behind its S half, so
                # the in-order PE never waits on an exp. ---
                b0q = []
                b0_state = {}

                def b0_attention_items():
                    items = []
                    for tsb in (3, 2, 1, 0):
                        g = tsb
                        n_sc = (tsb + 1) * 4
                        for h in range(HPC):
                            key = (g, h)
                            pend = []

                            def s_part(sc, g=g, h=h, tsb=tsb, key=key):
                                c0 = max(sc - tsb * 4, 0) * 128
                                if sc == 0:
                                    b0_state[key] = (
                                        PSL.tile([128, 512], f32, tag="lr",
                                                 name=f"b0av{g}_{h}"),
                                        LAP.tile([128, 512], fr, tag="pacc",
                                                 name=f"b0pacc{g}_{h}"))
                                st1 = PSST2.tile([128, 512], f32, tag="st2",
                                                 name=f"b0st{g}_{h}_{sc}")
                                pt1 = PTP.tile([128, 512], bf16, tag="pt2s",
                                               name=f"b0pt{g}_{h}_{sc}")
                                nc.tensor.matmul(
                                    st1[:, c0:512],
                                    lhsT=KTs[sc // 4][:, (sc % 4) * 128:
                                                      (sc % 4 + 1) * 128],
                                    rhs=QTs[h][g][:, c0:512],
                                    start=True, stop=True)
                                nc.scalar.activation(
                                    pt1[:, c0:512], st1[:, c0:512],
                                    mybir.ActivationFunctionType.Exp)
                                return pt1

                            def av_part(sc, pt1, g=g, h=h, tsb=tsb, key=key,
                                        n_sc=n_sc):
                                c0 = max(sc - tsb * 4, 0) * 128
                                av_ps, pacc = b0_state[key]
                                if sc >= tsb * 4:
                                    nc.gpsimd.affine_select(
                                        out=pt1[:, c0:c0 + 128],
                                        in_=pt1[:, c0:c0 + 128],
                                        compare_op=mybir.AluOpType.is_ge,
                                        fill=0.0,
                                        base=0,
                                        pattern=[[1, 128]],
                                        channel_multiplier=-1,
                                    )
                                if sc == 0:
                                    nc.gpsimd.tensor_copy(pacc[:, :], pt1[:, :])
                                else:
                                    nc.gpsimd.tensor_add(
                                        pacc[:, c0:512], pacc[:, c0:512],
                                        pt1[:, c0:512])
                                nc.tensor.matmul(
                                    av_ps[:, c0:512],
                                    lhsT=Vns[sc // 4][:, (sc % 4) * 128:
                                                      (sc % 4 + 1) * 128],
                                    rhs=pt1[:, c0:512],
                                    start=(sc == 0), stop=(sc == n_sc - 1))

                            def mk_s(sc, s_part=s_part, pend=pend):
                                def run(sc=sc, s_part=s_part, pend=pend):
                                    pt1 = s_part(sc)
                                    pend.append((sc, pt1))
                                return run

                            def mk_av(av_part=av_part, pend=pend):
                                def run(av_part=av_part, pend=pend):
                                    sc, pt1 = pend.pop(0)
                                    av_part(sc, pt1)
                                return run

                            def chain(g=g, h=h, key=key):
                                av_ps, pacc = b0_state[key]
                                nc.vector.tensor_copy(AVTs[h][g][:, :], av_ps[:])
                                l_bc = PSL.tile([128, 512], f32, tag="lr",
                                                name=f"b0l{g}_{h}")
                                nc.tensor.matmul(l_bc[:], lhsT=ones_sq[:],
                                                 rhs=pacc[:], start=True,
                                                 stop=True)
                                linv = RRP.tile([128, 512], bf16, tag="linv")
                                nc.vector.reciprocal(linv[:], l_bc[:])
                                nc.vector.tensor_mul(AVTs[h][g][:, :],
                                                     AVTs[h][g][:, :], linv[:])

                            items.append(mk_s(0))
                            for sc in range(1, n_sc):
                                items.append(mk_s(sc))
                                items.append(mk_av())
                            items.append(mk_av())
                            items.append(chain)
                    return items

                b0_points = [4 * 16]  # dci points in tq5 4..7
                def b0_drip():
                    if not b0q:
                        return
                    pts = b0_points[0]
                    k = -(-len(b0q) // max(pts, 1))
                    for _ in range(min(k, len(b0q))):
                        b0q.pop(0)()
                    b0_points[0] = max(pts - 1, 1)

                for tq5 in range(NG):
                    pq = [PSA.tile([128, 512], f32, tag=f"pq{h}", name=f"pq{h}") for h in range(HPC)]
                    pk = PSA.tile([128, 512], f32, tag="pk")
                    # all four V accumulators share one PSUM bank (PSUM
                    # slots are bank-granular; separate accumulation chains
                    # in disjoint column ranges of a bank are fine)
                    pv = PSA.tile([128, 512], f32, tag="pv")
                    xts = []
                    for dcg in range(4):
                        # one 512KB DMA: 4 d-chunks x 512 tokens
                        xt = XP.tile([128, 4 * 512], bf16, tag="x")
                        nc.sync.dma_start(
                            out=xt[:].rearrange("p (c m) -> p c m", c=4),
                            in_=xT[dcg * 512:(dcg + 1) * 512,
                                   tq5 * 512:(tq5 + 1) * 512].rearrange(
                                       "(c p) m -> p c m", p=128))
                        # tables + wo spread through tq5=1/2 x loads: the
                        # first rope (DVE) has ~100us of slack before its
                        # consumer, so keep the DMA queue clear for x tiles
                        if tq5 == 1 and dcg == 1:
                            nc.sync.dma_start(out=cos_sb[:], in_=cosT[:, :])
                        if tq5 == 1 and dcg == 3:
                            nc.sync.dma_start(out=nsin_sb[:], in_=nsinT[:, :])
                        if tq5 == 2 and dcg in (1, 3):
                            wh = dcg // 2
                            nc.sync.dma_start(
                                out=wo_sb[:, wh * D:(wh + 1) * D],
                                in_=woT[wh * 128:(wh + 1) * 128, :])
                        xts.append(xt)
                        for dci in range(4):
                            dc = dcg * 4 + dci
                            xs = xt[:, dci * 512:(dci + 1) * 512]
                            st, sp = (dc == 0), (dc == ND - 1)
                            for h in range(HPC):
                                nc.tensor.matmul(
                                    pq[h][:],
                                    lhsT=w_sb[:, dc * 512 + h * 128: dc * 512 + (h + 1) * 128],
                                    rhs=xs, start=st, stop=sp)
                            nc.tensor.matmul(
                                pk[:], lhsT=w_sb[:, dc * 512 + 256: dc * 512 + 384],
                                rhs=xs, start=st, stop=sp)
                            b0_drip()
                        # next weight quarter AFTER this dcg's matmuls:
                        # w_sb deps are whole-tile, so emitting the
                        # prefetch first would stall them on its DMA
                        if tq5 == 0 and dcg < 3:
                            load_weight_quarter(dcg + 1)
                    # V in natural [token, dh] layout, x-chunks as lhsT.
                    # One tb's 16-chunk chain finishes before the next
                    # starts: the four accumulators share one PSUM bank and
                    # a bank allows only one open accumulation group.
                    for tb in range(4):
                        for dc in range(ND):
                            xs = xts[dc // 4][:, (dc % 4) * 512:
                                              (dc % 4 + 1) * 512]
                            nc.tensor.matmul(
                                pv[:, tb * 128:(tb + 1) * 128],
                                lhsT=xs[:, tb * 128:(tb + 1) * 128],
                                rhs=w_sb[:, dc * 512 + 384: dc * 512 + 512],
                                start=(dc == 0), stop=(dc == ND - 1))
                        b0_drip()
                    # Vn copy first so the pv bank drains promptly
                    nc.vector.tensor_copy(Vns[tq5][:, :], pv[:, :])
                    for h in range(HPC):
                        nc.vector.tensor_copy(QTs[h][tq5][:, :], pq[h][:, :])
                    nc.vector.tensor_copy(KTs[tq5][:, :], pk[:, :])
                    # RoPE is deferred one tq5 iteration: its cos/nsin
                    # tables are DMA'd during tq5=1, and the tile framework
                    # orders deps by emission, so rope(k) is emitted at the
                    # end of iteration k+1 (consumers are ~100us away).
                    # HW cannot partition-shift in a 2-input op, but a
                    # 1-input DVE copy can: multiply by nsin in place
                    # (rows 0:64 = +sin, 64:128 = -sin), then swap halves
                    # with two shifted copies to build rot*sin.
                    def emit_rope(idx):
                        tab = slice((idx * 512) % T, (idx * 512) % T + 512)
                        for tgt in [KTs[idx], QTs[0][idx], QTs[1][idx]]:
                            rtmp = RT2.tile([128, 512], bf16, tag="rtmp")
                            nc.vector.tensor_mul(rtmp[:], tgt[:, :],
                                                 nsin_sb[:, tab])
                            rtsw = RT2.tile([128, 512], bf16, tag="rtsw")
                            nc.vector.tensor_copy(rtsw[0:64, :], rtmp[64:128, :])
                            nc.vector.tensor_copy(rtsw[64:128, :], rtmp[0:64, :])
                            nc.vector.tensor_mul(tgt[:, :], tgt[:, :],
                                                 cos_sb[:, tab])
                            nc.gpsimd.tensor_add(tgt[:, :], tgt[:, :], rtsw[:])
                    if 1 <= tq5 <= 2:
                        emit_rope(tq5 - 1)
                    if tq5 == 3:
                        emit_rope(2)
                        emit_rope(3)
                    if 5 <= tq5 <= 6:
                        emit_rope(tq5 - 1)
                    if tq5 == NG - 1:
                        emit_rope(tq5 - 1)
                        emit_rope(tq5)
                    if tq5 == 3:
                        b0q.extend(b0_attention_items())
                    if tq5 == NG - 1:
                        while b0q:
                            b0q.pop(0)()
              # ------- region 3: b=1 attention (paired exp) + ALL
              # groups' output projections -------
              with tc.tile_pool(name="psC", bufs=3, space="PSUM") as PSC, \
                 tc.tile_pool(name="psSt", bufs=2, space="PSUM") as PSST:
                osb_cnt = [0]

                def wo_units(b, tsb, fine=False):
                    # output projection for one 512-token group as 8
                    # independently emittable units (one osb half each),
                    # 512-wide d-quarters with double-buffered PSUM.
                    # fine=False DMAs each quarter separately (shorter tail).
                    g = b * NSB + tsb
                    units = []
                    for tj in range(4):
                        for dhalf in range(2):
                            def unit(tj=tj, dhalf=dhalf):
                                tcx = (b * T + tsb * 512) // 128 + tj
                                osb = OSBP.tile([128, D // 2], bf16, tag="osb")
                                for dq in range(2):
                                    dcol = dhalf * 1024 + dq * 512
                                    wo_ps = PSC.tile([128, 512], f32, tag="wops")
                                    for h in range(HPC):
                                        nc.tensor.matmul(
                                            wo_ps[:],
                                            lhsT=AVTs[h][g][:, tj * 128:(tj + 1) * 128],
                                            rhs=wo_sb[:, h * D + dcol: h * D + dcol + 512],
                                            start=(h == 0), stop=(h == HPC - 1))
                                    # only DVE and Act can read PSUM; Act
                                    # also runs exp, so weight DVE 2:1
                                    eng = osb_cnt[0] % 3
                                    osb_cnt[0] += 1
                                    dst = osb[:, dq * 512:(dq + 1) * 512]
                                    if eng == 2:
                                        nc.scalar.copy(dst, wo_ps[:])
                                    else:
                                        nc.vector.tensor_copy(dst, wo_ps[:])
                                    if fine:
                                        nc.sync.dma_start(
                                            out=out[tcx * 128:(tcx + 1) * 128,
                                                    dcol:dcol + 512],
                                            in_=dst)
                                if not fine:
                                    nc.sync.dma_start(
                                        out=out[tcx * 128:(tcx + 1) * 128,
                                                dhalf * 1024:(dhalf + 1) * 1024],
                                        in_=osb[:])
                            units.append(unit)
                    return units

                # b=0 attention already ran inside phase A; here the b=1
                # groups run with the b=0 groups' wo blocks as stall fill,
                # two wo groups flushed after each b=1 group's heads
                groups = [(1, 3), (1, 2), (1, 1), (1, 0)]
                flush_after = {(1, 3): [(0, 3), (0, 2)],
                               (1, 2): [(0, 1), (0, 0)],
                               (1, 1): [(1, 3)],
                               (1, 0): [(1, 2), (1, 1)]}
                def drip():
                    pass
                for b, tsb in groups:
                    n_sc = (tsb + 1) * 4
                    g = b * NSB + tsb
                    for h in range(HPC):
                        av_ps = PSL.tile([128, 512], f32, tag="lr",
                                         name="av_ps")
                        pacc = LAP.tile([128, 512], fr, tag="pacc")

                        def emit_av(pt2, halves):
                            # the P-consuming half of a pair: causal-mask
                            # select, denominator accumulate, AV matmuls
                            for sc, c0, off in halves:
                                # zero the upper triangle of the diagonal
                                # 128-block post-exp on Pool (exp of
                                # unmasked scores is bounded, ~exp(1.5))
                                if sc >= tsb * 4:
                                    nc.gpsimd.affine_select(
                                        out=pt2[:, off + c0: off + c0 + 128],
                                        in_=pt2[:, off + c0: off + c0 + 128],
                                        compare_op=mybir.AluOpType.is_ge,
                                        fill=0.0,
                                        base=0,
                                        pattern=[[1, 128]],
                                        channel_multiplier=-1,
                                    )
                                # softmax denominators accumulate on Pool
                                # (saves one PE column-stream per chunk)
                                if sc == 0:
                                    nc.gpsimd.tensor_copy(pacc[:, :], pt2[:, 0:512])
                                else:
                                    nc.gpsimd.tensor_add(
                                        pacc[:, c0:512], pacc[:, c0:512],
                                        pt2[:, off + c0: off + 512])
                                nc.tensor.matmul(
                                    av_ps[:, c0:512],
                                    lhsT=Vns[b * NSB + sc // 4][:, (sc % 4) * 128:
                                                                (sc % 4 + 1) * 128],
                                    rhs=pt2[:, off + c0: off + 512],
                                    start=(sc == 0), stop=(sc == n_sc - 1))

                        # lag-2 software pipeline: pair i's AV work is
                        # emitted after pair i+1's S/exp, so on the in-order
                        # PE the AV matmuls never wait on their own exp
                        avq = []
                        for scp in range(n_sc // 2):
                            st2 = PSST.tile([128, 1024], f32, tag="st")
                            pt2 = PTP.tile([128, 1024], bf16, tag="pt")
                            halves = []
                            for k in (0, 1):
                                sc = 2 * scp + k
                                c0 = max(sc - tsb * 4, 0) * 128
                                off = k * 512
                                nc.tensor.matmul(
                                    st2[:, off + c0: off + 512],
                                    lhsT=KTs[b * NSB + sc // 4][:, (sc % 4) * 128:
                                                                (sc % 4 + 1) * 128],
                                    rhs=QTs[h][g][:, c0:512],
                                    start=True, stop=True)
                                halves.append((sc, c0, off))
                            if halves[1][1] == 0:
                                # both halves full-width: one paired exp
                                nc.scalar.activation(
                                    pt2[:, :], st2[:, :],
                                    mybir.ActivationFunctionType.Exp)
                            else:
                                for sc, c0, off in halves:
                                    nc.scalar.activation(
                                        pt2[:, off + c0: off + 512],
                                        st2[:, off + c0: off + 512],
                                        mybir.ActivationFunctionType.Exp)
                            avq.append((pt2, halves))
                            if len(avq) > 1:
                                emit_av(*avq.pop(0))
                            drip()
                        for item in avq:
                            emit_av(*item)
                        # copy first: frees av_ps for the next head's
                        # accumulation while the l chain runs
                        nc.vector.tensor_copy(AVTs[h][g][:, :], av_ps[:])
                        # all-ones lhsT: one matmul both sums pacc over s
                        # AND broadcasts l to every partition
                        l_bc = PSL.tile([128, 512], f32, tag="lr", name="l_bc")
                        nc.tensor.matmul(l_bc[:], lhsT=ones_sq[:], rhs=pacc[:],
                                         start=True, stop=True)
                        linv = RRP.tile([128, 512], bf16, tag="linv")
                        nc.vector.reciprocal(linv[:], l_bc[:])
                        nc.vector.tensor_mul(AVTs[h][g][:, :], AVTs[h][g][:, :],
                                             linv[:])
                    for wg in flush_after[(b, tsb)]:
                        for u in wo_units(*wg):
                            u()
                for u in wo_units(1, 0, fine=False):
                    u()

    if split_waits:
        _split_multi_waits(nc, mybir)
    return nc


def _host_inputs(x, wq, wk, wv, wo):
    import ml_dtypes
    bf = ml_dtypes.bfloat16
    xT = np.ascontiguousarray(x.reshape(BT, D).T.astype(bf))
    half = DH // 2
    inv = (1.0 / (ROPE_BASE ** (np.arange(half, dtype=np.float32) / half))).astype(np.float32)
    ang = np.arange(T, dtype=np.float32)[:, None] * inv[None, :]          # (T, 64)
    c = np.cos(ang).T.astype(np.float32)                                  # (64, T)
    s = np.sin(ang).T.astype(np.float32)
    cosT = np.ascontiguousarray(np.concatenate([c, c], axis=0).astype(bf))  # (128, T)
    nsinT = np.ascontiguousarray(np.concatenate([s, -s], axis=0).astype(bf))
    scale = np.float32(1.0 / np.sqrt(DH))
    in_maps = []
    for core in range(NCORES):
        kvh = core // 2
        wqkvT = np.concatenate([
            (wq[core * HPC * DH:(core + 1) * HPC * DH, :] * scale).T,
            wk[kvh * DH:(kvh + 1) * DH, :].T,
            wv[kvh * DH:(kvh + 1) * DH, :].T,
        ], axis=1)
        in_maps.append({
            "xT": xT,
            "wqkvT": np.ascontiguousarray(wqkvT.astype(bf)),
            "woT": np.ascontiguousarray(wo[:, core * HPC * DH:(core + 1) * HPC * DH].T.astype(bf)),
            "cosT": cosT,
            "nsinT": nsinT,
        })
    return in_maps


def kernel(x, wq, wk, wv, wo):
    _ensure_path()
    from concourse.bass_utils import run_bass_kernel_spmd

    x = np.asarray(x, dtype=np.float32)
    wq = np.asarray(wq, dtype=np.float32)
    wk = np.asarray(wk, dtype=np.float32)
    wv = np.asarray(wv, dtype=np.float32)
    wo = np.asarray(wo, dtype=np.float32)

    if "nc" not in _cache:
        _cache["nc"] = _build()
    nc = _cache["nc"]

    in_maps = _host_inputs(x, wq, wk, wv, wo)
    res = run_bass_kernel_spmd(nc, in_maps, list(range(NCORES)))
    acc = np.asarray(res.results[0]["out"]).astype(np.float32)
    for cidx in range(1, NCORES):
        acc = acc + np.asarray(res.results[cidx]["out"]).astype(np.float32)
    return acc.reshape(B, T, D)


# revision 59
# speedup vs baseline: 1.3680x; 1.0028x over previous
"""Tensor-parallel GQA multi-head attention for 8 Trainium2 NeuronCores.

Sharding: query heads (16) split 2-per-core; each core needs exactly one
KV head (GQA group); wq/wk/wv column-parallel, wo row-parallel; the
all-reduce after wo is done host-side (sum of 8 partial outputs).

Per-core dataflow (all matmul operands bf16, PSUM f32):
  QT/KT = Wqk^T-chunks (lhsT) x xT (rhs)       [dh, tokens]
  Vn    = xT-chunks (lhsT) x Wv^T (rhs)        [tokens, dh] natural layout
  rope  on QT/KT on DVE: mul by [sin;-sin], swap halves with two
          partition-shifted copies (1-input copies may shift partitions
          on HW; 2-input ops may not), mul by cos, add on Pool
  S^T   = KT-chunk (lhsT) x QT (rhs)           [s, t]  (causal superblocks)
  P^T   = exp(S^T)                             (no max-subtraction: scores
                                                are bounded ~N(0, 1/9));
          causal mask = Pool affine_select zeroing the diag upper triangle
  l     = Pool-accumulated sum of P^T chunks, then one all-ones matmul
          that both reduces and broadcasts [128, t]; 1/l applied on DVE
  avT   = Vn-chunk (lhsT) x P^T (rhs)          [dh, t]; scaled by 1/l
  out   = avT-chunk (lhsT) x woT (rhs)         [t, d] partial, bf16 DMA out

Schedule: three regions, PSUM exactly 8 banks each.
 1. tq5=0..3 projections (Q/K per d-chunk; V as four sequential
    per-token-block chains sharing one PSUM bank).
 2. tq5=4..7 projections with the b=0 attention groups micro-interleaved
    between projection chunks (one S+exp, one mask+denom+AV, or one
    normalization chain per slot) so the in-order PE never stalls on the
    softmax chain; av/l_bc share one PSUM bank (their chains alternate).
 3. b=1 attention with paired 1024-wide exp super-tiles, plus all eight
    groups' output projections as fill between/after them, wo quarters
    triple-buffered in PSUM.
Q/K/V and AV live in per-512-token-superblock tiles so whole-tile
dependency tracking never serializes attention behind later projections.
"""

import numpy as np

B, T, D, H, KV = 2, 2048, 2048, 16, 4
DH = 128
NCORES = 8
HPC = H // NCORES          # 2 query heads per core
BT = B * T                 # 4096
ND = D // 128              # 16 contraction chunks
NSB = T // 512             # 4 causal superblocks per batch
NG = B * NSB               # 8 (batch, superblock) groups
ROPE_BASE = 10000.0
NEG = -1.0e4
_WO_RESERVE = 8            # wo units kept for the group-end flush
_N_WARM = 4                # keep-warm matmuls before the first projection

_cache = {}


def _ensure_path():
    try:
        import concourse.bass  # noqa: F401
    except ImportError:
        import sys
        for p in ("/opt/trn_rl_repo", "/root/.axon_site/_ro/trn_rl_repo"):
            if p not in sys.path:
                sys.path.insert(0, p)
        import concourse.bass  # noqa: F401


def _split_multi_waits(nc, mybir, max_waits=1):
    """This container's walrus rejects >1 sync-wait on one instruction
    (seen on the Tile tail drain). Move extra waits onto preceding NoOps
    on the same engine; per-engine program order preserves semantics."""
    for bb in nc.main_func.blocks:
        new_insts = []
        for ins in bb.instructions:
            si = getattr(ins, "sync_info", None)
            if si is not None and si.on_wait and len(si.on_wait) > max_waits:
                waits = list(si.on_wait)
                extra, keep = waits[:-max_waits], waits[-max_waits:]
                for w in extra:
                    new_insts.append(
                        mybir.InstNoOp(
                            name=nc.get_next_instruction_name(),
                            sync_info=mybir.SyncInfo(on_wait=[w], on_update=[]),
                            bass_nofuse=True,
                            engine=ins.engine,
                            ins=[],
                            outs=[],
                        )
                    )
                si.on_wait = keep
            new_insts.append(ins)
        bb.instructions = new_insts


def _build(split_waits=True):
    _ensure_path()
    import concourse.bass as bass
    import concourse.mybir as mybir
    import concourse.tile as tile
    from concourse.masks import make_identity

    f32 = mybir.dt.float32
    fr = mybir.dt.float32r
    bf16 = mybir.dt.bfloat16
    nc = bass.Bass()

    xT = nc.declare_dram_parameter("xT", [D, BT], bf16, isOutput=False)
    # per 128-row chunk c: cols 0:128 q head0, 128:256 q head1, 256:384 k,
    # 384:512 v
    wqkvT = nc.declare_dram_parameter("wqkvT", [D, 4 * DH], bf16, isOutput=False)
    woT = nc.declare_dram_parameter("woT", [HPC * DH, D], bf16, isOutput=False)
    cosT = nc.declare_dram_parameter("cosT", [DH, T], bf16, isOutput=False)
    # rows 0:64 hold +sin, rows 64:128 hold -sin
    nsinT = nc.declare_dram_parameter("nsinT", [DH, T], bf16, isOutput=False)
    out = nc.declare_dram_parameter("out", [BT, D], bf16, isOutput=True)

    with nc.allow_low_precision(reason="bf16 fast matmul path"), \
         tile.TileContext(nc) as tc:
        with tc.tile_pool(name="persist", bufs=1) as P:
            ident = P.tile([128, 128], f32, tag="ident")
            ones_sq = P.tile([128, 128], fr, tag="ones_sq")
            ones_sqf = P.tile([128, 128], f32, tag="ones_sqf")
            ones_rf = P.tile([1, 128], f32, tag="ones_rf")
            make_identity(nc, ident[:])
            nc.gpsimd.memset(ones_sqf[:], 1.0)
            nc.vector.tensor_copy(ones_sq[:], ones_sqf[:])
            nc.gpsimd.memset(ones_rf[:], 1.0)
            # dummy exp: pull the 1.3us activation-table load into phase A
            # instead of paying it on the first real softmax exp
            dummy_e = P.tile([1, 1], f32, tag="dummy_e")
            nc.scalar.activation(dummy_e[:], ones_rf[0:1, 0:1],
                                 mybir.ActivationFunctionType.Exp)

            cos_sb = P.tile([128, T], bf16, tag="cos")
            nsin_sb = P.tile([128, T], bf16, tag="nsin")
            wo_sb = P.tile([128, HPC * D], bf16, tag="wo")
            # per-superblock tiles: fine-grained deps let attention start
            # on a superblock as soon as its projections+rope finish
            QTs = [[P.tile([128, 512], bf16, tag=f"qt{h}_{g}", name=f"qt{h}_{g}")
                    for g in range(NG)] for h in range(HPC)]
            KTs = [P.tile([128, 512], bf16, tag=f"kt{g}", name=f"kt{g}")
                   for g in range(NG)]
            Vns = [P.tile([128, 512], bf16, tag=f"vn{g}", name=f"vn{g}")
                   for g in range(NG)]
            AVTs = [[P.tile([128, 512], bf16, tag=f"avt{h}_{g}", name=f"avt{h}_{g}")
                     for g in range(NG)] for h in range(HPC)]

            # pools shared by both regions. PSUM budget is exact 16KB
            # per partition in each region:
            #   region A/2: psA 8KB + psSt2 4KB + psAv 2KB + psL 2KB
            #   region 3:   psSt3 8KB + psC 4KB + psAv 2KB + psL 2KB
            with tc.tile_pool(name="ptp", bufs=6) as PTP, \
                 tc.tile_pool(name="lap", bufs=3) as LAP, \
                 tc.tile_pool(name="rrp", bufs=2) as RRP, \
                 tc.tile_pool(name="osbp", bufs=4) as OSBP, \
                 tc.tile_pool(name="psL", bufs=1, space="PSUM") as PSL:
              # ------- phase A: QKV projections + RoPE, with the b=0
              # attention groups micro-interleaved into tq5>=4 so the
              # projection stream hides their exp-chain latency -------
              with tc.tile_pool(name="wpool", bufs=1) as WP, \
                 tc.tile_pool(name="xp", bufs=6) as XP, \
                 tc.tile_pool(name="ropetA", bufs=4) as RT2, \
                 tc.tile_pool(name="psA", bufs=1, space="PSUM") as PSA, \
                 tc.tile_pool(name="psSt2", bufs=3, space="PSUM") as PSST2:
                w_sb = WP.tile([128, ND * 4 * DH], bf16, tag="w")
                def load_weight_quarter(qi):
                    lo, hi = qi * (ND // 4), (qi + 1) * (ND // 4)
                    nc.sync.dma_start(
                        out=w_sb[:, lo * 512: hi * 512].rearrange(
                            "p (c m) -> p c m", c=hi - lo),
                        in_=wqkvT[lo * 128: hi * 128, :].rearrange(
                            "(c p) m -> p c m", p=128))

                # keep-warm matmuls: the PE would otherwise sit idle for
                # the initial weight/x DMA and pay the HAM half-clock ramp
                # when the first projections finally issue.
                for _ in range(_N_WARM):
                    wps = PSA.tile([128, 512], f32, tag="pv", name="warm")
                    nc.tensor.matmul(wps[:, 0:128], lhsT=ident[:], rhs=ident[:],
                                     start=True, stop=True)

                # only the first quarter of the weights before the first x
                # tile; the rest interleave with tq5=0's x loads so the first
                # matmuls start earlier
                load_weight_quarter(0)

                # --- b=0 attention as a queue of micro-ops (one S+exp or
                # one mask+denominator+AV or one normalization chain each),
                # popped between projection chunks of tq5>=4. Items are
                # ordered with the AV half one slot behind its S half, so
                # the in-order PE never waits on an exp. ---
                b0q = []
                b0_state = {}

                def b0_attention_items():
                    items = []
                    for tsb in (3, 2, 1, 0):
                        g = tsb
                        n_sc = (tsb + 1) * 4
                        for h in range(HPC):
                            key = (g, h)
                            pend = []

                            def s_part(sc, g=g, h=h, tsb=tsb, key=key):
                                c0 = max(sc - tsb * 4, 0) * 128
                                if sc == 0:
                                    b0_state[key] = (
                                        PSL.tile([128, 512], f32, tag="lr",
                                                 name=f"b0av{g}_{h}"),
                                        LAP.tile([128, 512], fr, tag="pacc",
                                                 name=f"b0pacc{g}_{h}"))
                                st1 = PSST2.tile([128, 512], f32, tag="st2",
                                                 name=f"b0st{g}_{h}_{sc}")
                                pt1 = PTP.tile([128, 512], bf16, tag="pt2s",
                                               name=f"b0pt{g}_{h}_{sc}")
                                nc.tensor.matmul(
                                    st1[:, c0:512],
                                    lhsT=KTs[sc // 4][:, (sc % 4) * 128:
                                                      (sc % 4 + 1) * 128],
                                    rhs=QTs[h][g][:, c0:512],
                                    start=True, stop=True)
                                nc.scalar.activation(
                                    pt1[:, c0:512], st1[:, c0:512],
                                    mybir.ActivationFunctionType.Exp)
                                return pt1

                            def av_part(sc, pt1, g=g, h=h, tsb=tsb, key=key,
                                        n_sc=n_sc):
                                c0 = max(sc - tsb * 4, 0) * 128
                                av_ps, pacc = b0_state[key]
                                if sc >= tsb * 4:
                                    nc.gpsimd.affine_select(
                                        out=pt1[:, c0:c0 + 128],
                                        in_=pt1[:, c0:c0 + 128],
                                        compare_op=mybir.AluOpType.is_ge,
                                        fill=0.0,
                                        base=0,
                                        pattern=[[1, 128]],
                                        channel_multiplier=-1,
                                    )
                                if sc == 0:
                                    nc.gpsimd.tensor_copy(pacc[:, :], pt1[:, :])
                                else:
                                    nc.gpsimd.tensor_add(
                                        pacc[:, c0:512], pacc[:, c0:512],
                                        pt1[:, c0:512])
                                nc.tensor.matmul(
                                    av_ps[:, c0:512],
                                    lhsT=Vns[sc // 4][:, (sc % 4) * 128:
                                                      (sc % 4 + 1) * 128],
                                    rhs=pt1[:, c0:512],
                                    start=(sc == 0), stop=(sc == n_sc - 1))

                            def mk_s(sc, s_part=s_part, pend=pend):
                                def run(sc=sc, s_part=s_part, pend=pend):
                                    pt1 = s_part(sc)
                                    pend.append((sc, pt1))
                                return run

                            def mk_av(av_part=av_part, pend=pend):
                                def run(av_part=av_part, pend=pend):
                                    sc, pt1 = pend.pop(0)
                                    av_part(sc, pt1)
                                return run

                            def chain(g=g, h=h, key=key):
                                av_ps, pacc = b0_state[key]
                                nc.vector.tensor_copy(AVTs[h][g][:, :], av_ps[:])
                                l_bc = PSL.tile([128, 512], f32, tag="lr",
                                                name=f"b0l{g}_{h}")
                                nc.tensor.matmul(l_bc[:], lhsT=ones_sq[:],
                                                 rhs=pacc[:], start=True,
                                                 stop=True)
                                linv = RRP.tile([128, 512], bf16, tag="linv")
                                nc.vector.reciprocal(linv[:], l_bc[:])
                                nc.vector.tensor_mul(AVTs[h][g][:, :],
                                                     AVTs[h][g][:, :], linv[:])

                            items.append(mk_s(0))
                            for sc in range(1, n_sc):
                                items.append(mk_s(sc))
                                items.append(mk_av())
                            items.append(mk_av())
                            items.append(chain)
                    return items

                b0_points = [4 * 16]  # dci points in tq5 4..7
                def b0_drip():
                    if not b0q:
                        return
                    pts = b0_points[0]
                    k = -(-len(b0q) // max(pts, 1))
                    for _ in range(min(k, len(b0q))):
                        b0q.pop(0)()
                    b0_points[0] = max(pts - 1, 1)

                for tq5 in range(NG):
                    pq = [PSA.tile([128, 512], f32, tag=f"pq{h}", name=f"pq{h}") for h in range(HPC)]
                    pk = PSA.tile([128, 512], f32, tag="pk")
                    # all four V accumulators share one PSUM bank (PSUM
                    # slots are bank-granular; separate accumulation chains
                    # in disjoint column ranges of a bank are fine)
                    pv = PSA.tile([128, 512], f32, tag="pv")
                    xts = []
                    for dcg in range(4):
                        # one 512KB DMA: 4 d-chunks x 512 tokens
                        xt = XP.tile([128, 4 * 512], bf16, tag="x")
                        nc.sync.dma_start(
                            out=xt[:].rearrange("p (c m) -> p c m", c=4),
                            in_=xT[dcg * 512:(dcg + 1) * 512,
                                   tq5 * 512:(tq5 + 1) * 512].rearrange(
                                       "(c p) m -> p c m", p=128))
                        # tables + wo spread through tq5=1/2 x loads: the
                        # first rope (DVE) has ~100us of slack before its
                        # consumer, so keep the DMA queue clear for x tiles
                        if tq5 == 1 and dcg == 1:
                            nc.sync.dma_start(out=cos_sb[:], in_=cosT[:, :])
                        if tq5 == 1 and dcg == 3:
                            nc.sync.dma_start(out=nsin_sb[:], in_=nsinT[:, :])
                        if tq5 == 2 and dcg in (1, 3):
                            wh = dcg // 2
                            nc.sync.dma_start(
                                out=wo_sb[:, wh * D:(wh + 1) * D],
                                in_=woT[wh * 128:(wh + 1) * 128, :])
                        xts.append(xt)
                        for dci in range(4):
                            dc = dcg * 4 + dci
                            xs = xt[:, dci * 512:(dci + 1) * 512]
                            st, sp = (dc == 0), (dc == ND - 1)
                            for h in range(HPC):
                                nc.tensor.matmul(
                                    pq[h][:],
                                    lhsT=w_sb[:, dc * 512 + h * 128: dc * 512 + (h + 1) * 128],
                                    rhs=xs, start=st, stop=sp)
                            nc.tensor.matmul(
                                pk[:], lhsT=w_sb[:, dc * 512 + 256: dc * 512 + 384],
                                rhs=xs, start=st, stop=sp)
                            b0_drip()
                        # next weight quarter AFTER this dcg's matmuls:
                        # w_sb deps are whole-tile, so emitting the
                        # prefetch first would stall them on its DMA
                        if tq5 == 0 and dcg < 3:
                            load_weight_quarter(dcg + 1)
                    # V in natural [token, dh] layout, x-chunks as lhsT.
                    # One tb's 16-chunk chain finishes before the next
                    # starts: the four accumulators share one PSUM bank and
                    # a bank allows only one open accumulation group.
                    for tb in range(4):
                        for dc in range(ND):
                            xs = xts[dc // 4][:, (dc % 4) * 512:
                                              (dc % 4 + 1) * 512]
                            nc.tensor.matmul(
                                pv[:, tb * 128:(tb + 1) * 128],
                                lhsT=xs[:, tb * 128:(tb + 1) * 128],
                                rhs=w_sb[:, dc * 512 + 384: dc * 512 + 512],
                                start=(dc == 0), stop=(dc == ND - 1))
                        b0_drip()
                    # Vn copy first so the pv bank drains promptly
                    nc.vector.tensor_copy(Vns[tq5][:, :], pv[:, :])
                    for h in range(HPC):
                        nc.vector.tensor_copy(QTs[h][tq5][:, :], pq[h][:, :])
                    nc.vector.tensor_copy(KTs[tq5][:, :], pk[:, :])
                    # RoPE is deferred one tq5 iteration: its cos/nsin
                    # tables are DMA'd during tq5=1, and the tile framework
                    # orders deps by emission, so rope(k) is emitted at the
                    # end of iteration k+1 (consumers are ~100us away).
                    # HW cannot partition-shift in a 2-input op, but a
                    # 1-input DVE copy can: multiply by nsin in place
                    # (rows 0:64 = +sin, 64:128 = -sin), then swap halves
                    # with two shifted copies to build rot*sin.
                    def emit_rope(idx):
                        tab = slice((idx * 512) % T, (idx * 512) % T + 512)
                        for tgt in [KTs[idx], QTs[0][idx], QTs[1][idx]]:
                            rtmp = RT2.tile([128, 512], bf16, tag="rtmp")
                            nc.vector.tensor_mul(rtmp[:], tgt[:, :],
                                                 nsin_sb[:, tab])
                            rtsw = RT2.tile([128, 512], bf16, tag="rtsw")
                            nc.vector.tensor_copy(rtsw[0:64, :], rtmp[64:128, :])
                            nc.vector.tensor_copy(rtsw[64:128, :], rtmp[0:64, :])
                            nc.vector.tensor_mul(tgt[:, :], tgt[:, :],
                                                 cos_sb[:, tab])
                            nc.gpsimd.tensor_add(tgt[:, :], tgt[:, :], rtsw[:])
                    if 1 <= tq5 <= 2:
                        emit_rope(tq5 - 1)
                    if tq5 == 3:
                        emit_rope(2)
                        emit_rope(3)
                    if 5 <= tq5 <= 6:
                        emit_rope(tq5 - 1)
                    if tq5 == NG - 1:
                        emit_rope(tq5 - 1)
                        emit_rope(tq5)
                    if tq5 == 3:
                        b0q.extend(b0_attention_items())
                    if tq5 == NG - 1:
                        while b0q:
                            b0q.pop(0)()
              # ------- region 3: b=1 attention (paired exp) + ALL
              # groups' output projections -------
              with tc.tile_pool(name="psC", bufs=3, space="PSUM") as PSC, \
                 tc.tile_pool(name="psSt", bufs=2, space="PSUM") as PSST:
                osb_cnt = [0]

                def wo_units(b, tsb, fine=False):
                    # output projection for one 512-token group as 8
                    # independently emittable units (one osb half each),
                    # 512-wide d-quarters with double-buffered PSUM.
                    # fine=False DMAs each quarter separately (shorter tail).
                    g = b * NSB + tsb
                    units = []
                    for tj in range(4):
                        for dhalf in range(2):
                            def unit(tj=tj, dhalf=dhalf):
                                tcx = (b * T + tsb * 512) // 128 + tj
                                osb = OSBP.tile([128, D // 2], bf16, tag="osb")
                                for dq in range(2):
                                    dcol = dhalf * 1024 + dq * 512
                                    wo_ps = PSC.tile([128, 512], f32, tag="wops")
                                    for h in range(HPC):
                                        nc.tensor.matmul(
                                            wo_ps[:],
                                            lhsT=AVTs[h][g][:, tj * 128:(tj + 1) * 128],
                                            rhs=wo_sb[:, h * D + dcol: h * D + dcol + 512],
                                            start=(h == 0), stop=(h == HPC - 1))
                                    # only DVE and Act can read PSUM; Act
                                    # also runs exp, so weight DVE 2:1
                                    eng = osb_cnt[0] % 3
                                    osb_cnt[0] += 1
                                    dst = osb[:, dq * 512:(dq + 1) * 512]
                                    if eng == 2:
                                        nc.scalar.copy(dst, wo_ps[:])
                                    else:
                                        nc.vector.tensor_copy(dst, wo_ps[:])
                                    if fine:
                                        nc.sync.dma_start(
                                            out=out[tcx * 128:(tcx + 1) * 128,
                                                    dcol:dcol + 512],
                                            in_=dst)
                                if not fine:
                                    nc.sync.dma_start(
                                        out=out[tcx * 128:(tcx + 1) * 128,
                                                dhalf * 1024:(dhalf + 1) * 1024],
                                        in_=osb[:])
                            units.append(unit)
                    return units

                # b=0 attention already ran inside phase A; here the b=1
                # groups run with the b=0 groups' wo blocks as stall fill,
                # two wo groups flushed after each b=1 group's heads
                # ordered by rope availability: (1,1) needs only
                # rope(4,5); (1,3) (which needs rope(7), emitted last)
                # runs third
                groups = [(1, 1), (1, 2), (1, 3), (1, 0)]
                flush_after = {(1, 1): [(0, 3), (0, 2)],
                               (1, 2): [(0, 1), (0, 0)],
                               (1, 3): [(1, 1), (1, 2)],
                               (1, 0): [(1, 3)]}
                def drip():
                    pass
                for b, tsb in groups:
                    n_sc = (tsb + 1) * 4
                    g = b * NSB + tsb
                    for h in range(HPC):
                        av_ps = PSL.tile([128, 512], f32, tag="lr",
                                         name="av_ps")
                        pacc = LAP.tile([128, 512], fr, tag="pacc")

                        def emit_av(pt2, halves):
                            # the P-consuming half of a pair: causal-mask
                            # select, denominator accumulate, AV matmuls
                            for sc, c0, off in halves:
                                # zero the upper triangle of the diagonal
                                # 128-block post-exp on Pool (exp of
                                # unmasked scores is bounded, ~exp(1.5))
                                if sc >= tsb * 4:
                                    nc.gpsimd.affine_select(
                                        out=pt2[:, off + c0: off + c0 + 128],
                                        in_=pt2[:, off + c0: off + c0 + 128],
                                        compare_op=mybir.AluOpType.is_ge,
                                        fill=0.0,
                                        base=0,
                                        pattern=[[1, 128]],
                                        channel_multiplier=-1,
                                    )
                                # softmax denominators accumulate on Pool
                                # (saves one PE column-stream per chunk)
                                if sc == 0:
                                    nc.gpsimd.tensor_copy(pacc[:, :], pt2[:, 0:512])
                                else:
                                    nc.gpsimd.tensor_add(
                                        pacc[:, c0:512], pacc[:, c0:512],
                                        pt2[:, off + c0: off + 512])
                                nc.tensor.matmul(
                                    av_ps[:, c0:512],
                                    lhsT=Vns[b * NSB + sc // 4][:, (sc % 4) * 128:
                                                                (sc % 4 + 1) * 128],
                                    rhs=pt2[:, off + c0: off + 512],
                                    start=(sc == 0), stop=(sc == n_sc - 1))

                        # lag-2 software pipeline: pair i's AV work is
                        # emitted after pair i+1's S/exp, so on the in-order
                        # PE the AV matmuls never wait on their own exp
                        avq = []
                        for scp in range(n_sc // 2):
                            st2 = PSST.tile([128, 1024], f32, tag="st")
                            pt2 = PTP.tile([128, 1024], bf16, tag="pt")
                            halves = []
                            for k in (0, 1):
                                sc = 2 * scp + k
                                c0 = max(sc - tsb * 4, 0) * 128
                                off = k * 512
                                nc.tensor.matmul(
                                    st2[:, off + c0: off + 512],
                                    lhsT=KTs[b * NSB + sc // 4][:, (sc % 4) * 128:
                                                                (sc % 4 + 1) * 128],
                                    rhs=QTs[h][g][:, c0:512],
                                    start=True, stop=True)
                                halves.append((sc, c0, off))
                            if halves[1][1] == 0:
                                # both halves full-width: one paired exp
                                nc.scalar.activation(
                                    pt2[:, :], st2[:, :],
                                    mybir.ActivationFunctionType.Exp)
                            else:
                                for sc, c0, off in halves:
                                    nc.scalar.activation(
                                        pt2[:, off + c0: off + 512],
                                        st2[:, off + c0: off + 512],
                                        mybir.ActivationFunctionType.Exp)
                            avq.append((pt2, halves))
                            if len(avq) > 1:
                                emit_av(*avq.pop(0))
                            drip()
                        for item in avq:
                            emit_av(*item)
                        # copy first: frees av_ps for the next head's
                        # accumulation while the l chain runs
                        nc.vector.tensor_copy(AVTs[h][g][:, :], av_ps[:])
                        # all-ones lhsT: one matmul both sums pacc over s
                        # AND broadcasts l to every partition
                        l_bc = PSL.tile([128, 512], f32, tag="lr", name="l_bc")
                        nc.tensor.matmul(l_bc[:], lhsT=ones_sq[:], rhs=pacc[:],
                                         start=True, stop=True)
                        linv = RRP.tile([128, 512], bf16, tag="linv")
                        nc.vector.reciprocal(linv[:], l_bc[:])
                        nc.vector.tensor_mul(AVTs[h][g][:, :], AVTs[h][g][:, :],
                                             linv[:])
                    for wg in flush_after[(b, tsb)]:
                        for u in wo_units(*wg):
                            u()
                for u in wo_units(1, 0, fine=False):
                    u()

    if split_waits:
        _split_multi_waits(nc, mybir)
    return nc


def _host_inputs(x, wq, wk, wv, wo):
    import ml_dtypes
    bf = ml_dtypes.bfloat16
    xT = np.ascontiguousarray(x.reshape(BT, D).T.astype(bf))
    half = DH // 2
    inv = (1.0 / (ROPE_BASE ** (np.arange(half, dtype=np.float32) / half))).astype(np.float32)
    ang = np.arange(T, dtype=np.float32)[:, None] * inv[None, :]          # (T, 64)
    c = np.cos(ang).T.astype(np.float32)                                  # (64, T)
    s = np.sin(ang).T.astype(np.float32)
    cosT = np.ascontiguousarray(np.concatenate([c, c], axis=0).astype(bf))  # (128, T)
    nsinT = np.ascontiguousarray(np.concatenate([s, -s], axis=0).astype(bf))
    scale = np.float32(1.0 / np.sqrt(DH))
    in_maps = []
    for core in range(NCORES):
        kvh = core // 2
        wqkvT = np.concatenate([
            (wq[core * HPC * DH:(core + 1) * HPC * DH, :] * scale).T,
            wk[kvh * DH:(kvh + 1) * DH, :].T,
            wv[kvh * DH:(kvh + 1) * DH, :].T,
        ], axis=1)
        in_maps.append({
            "xT": xT,
            "wqkvT": np.ascontiguousarray(wqkvT.astype(bf)),
            "woT": np.ascontiguousarray(wo[:, core * HPC * DH:(core + 1) * HPC * DH].T.astype(bf)),
            "cosT": cosT,
            "nsinT": nsinT,
        })
    return in_maps


def kernel(x, wq, wk, wv, wo):
    _ensure_path()
    from concourse.bass_utils import run_bass_kernel_spmd

    x = np.asarray(x, dtype=np.float32)
    wq = np.asarray(wq, dtype=np.float32)
    wk = np.asarray(wk, dtype=np.float32)
    wv = np.asarray(wv, dtype=np.float32)
    wo = np.asarray(wo, dtype=np.float32)

    if "nc" not in _cache:
        _cache["nc"] = _build()
    nc = _cache["nc"]

    in_maps = _host_inputs(x, wq, wk, wv, wo)
    res = run_bass_kernel_spmd(nc, in_maps, list(range(NCORES)))
    acc = np.asarray(res.results[0]["out"]).astype(np.float32)
    for cidx in range(1, NCORES):
        acc = acc + np.asarray(res.results[cidx]["out"]).astype(np.float32)
    return acc.reshape(B, T, D)


# revision 78
# speedup vs baseline: 1.3709x; 1.0022x over previous
"""Tensor-parallel GQA multi-head attention for 8 Trainium2 NeuronCores.

Sharding: query heads (16) split 2-per-core; each core needs exactly one
KV head (GQA group); wq/wk/wv column-parallel, wo row-parallel; the
all-reduce after wo is done host-side (sum of 8 partial outputs).

Per-core dataflow (all matmul operands bf16, PSUM f32):
  QT/KT = Wqk^T-chunks (lhsT) x xT (rhs)       [dh, tokens]
  Vn    = xT-chunks (lhsT) x Wv^T (rhs)        [tokens, dh] natural layout
  rope  on QT/KT on DVE: mul by [sin;-sin], swap halves with two
          partition-shifted copies (1-input copies may shift partitions
          on HW; 2-input ops may not), mul by cos, add on Pool
  S^T   = KT-chunk (lhsT) x QT (rhs)           [s, t]  (causal superblocks)
  P^T   = exp(S^T)                             (no max-subtraction: scores
                                                are bounded ~N(0, 1/9));
          causal mask = Pool affine_select zeroing the diag upper triangle
  l     = Pool-accumulated sum of P^T chunks, then one all-ones matmul
          that both reduces and broadcasts [128, t]; 1/l applied on DVE
  avT   = Vn-chunk (lhsT) x P^T (rhs)          [dh, t]; scaled by 1/l
  out   = avT-chunk (lhsT) x woT (rhs)         [t, d] partial, bf16 DMA out

Schedule: three regions, PSUM exactly 8 banks each.
 1. tq5=0..3 projections (Q/K per d-chunk; V as four sequential
    per-token-block chains sharing one PSUM bank).
 2. tq5=4..7 projections with the b=0 attention groups micro-interleaved
    between projection chunks (one S+exp, one mask+denom+AV, or one
    normalization chain per slot) so the in-order PE never stalls on the
    softmax chain; av/l_bc share one PSUM bank (their chains alternate).
 3. b=1 attention with paired 1024-wide exp super-tiles, plus all eight
    groups' output projections as fill between/after them, wo quarters
    triple-buffered in PSUM.
Q/K/V and AV live in per-512-token-superblock tiles so whole-tile
dependency tracking never serializes attention behind later projections.
"""

import numpy as np

B, T, D, H, KV = 2, 2048, 2048, 16, 4
DH = 128
NCORES = 8
HPC = H // NCORES          # 2 query heads per core
BT = B * T                 # 4096
ND = D // 128              # 16 contraction chunks
NSB = T // 512             # 4 causal superblocks per batch
NG = B * NSB               # 8 (batch, superblock) groups
ROPE_BASE = 10000.0
NEG = -1.0e4
_WO_RESERVE = 8            # wo units kept for the group-end flush
_N_WARM = 4                # keep-warm matmuls before the first projection

_cache = {}


def _ensure_path():
    try:
        import concourse.bass  # noqa: F401
    except ImportError:
        import sys
        for p in ("/opt/trn_rl_repo", "/root/.axon_site/_ro/trn_rl_repo"):
            if p not in sys.path:
                sys.path.insert(0, p)
        import concourse.bass  # noqa: F401


def _split_multi_waits(nc, mybir, max_waits=1):
    """This container's walrus rejects >1 sync-wait on one instruction
    (seen on the Tile tail drain). Move extra waits onto preceding NoOps
    on the same engine; per-engine program order preserves semantics."""
    for bb in nc.main_func.blocks:
        new_insts = []
        for ins in bb.instructions:
            si = getattr(ins, "sync_info", None)
            if si is not None and si.on_wait and len(si.on_wait) > max_waits:
                waits = list(si.on_wait)
                extra, keep = waits[:-max_waits], waits[-max_waits:]
                for w in extra:
                    new_insts.append(
                        mybir.InstNoOp(
                            name=nc.get_next_instruction_name(),
                            sync_info=mybir.SyncInfo(on_wait=[w], on_update=[]),
                            bass_nofuse=True,
                            engine=ins.engine,
                            ins=[],
                            outs=[],
                        )
                    )
                si.on_wait = keep
            new_insts.append(ins)
        bb.instructions = new_insts


def _build(split_waits=True):
    _ensure_path()
    import concourse.bass as bass
    import concourse.mybir as mybir
    import concourse.tile as tile
    from concourse.masks import make_identity

    f32 = mybir.dt.float32
    fr = mybir.dt.float32r
    bf16 = mybir.dt.bfloat16
    nc = bass.Bass()

    xT = nc.declare_dram_parameter("xT", [D, BT], bf16, isOutput=False)
    # per 128-row chunk c: cols 0:128 q head0, 128:256 q head1, 256:384 k,
    # 384:512 v
    wqkvT = nc.declare_dram_parameter("wqkvT", [D, 4 * DH], bf16, isOutput=False)
    woT = nc.declare_dram_parameter("woT", [HPC * DH, D], bf16, isOutput=False)
    cosT = nc.declare_dram_parameter("cosT", [DH, T], bf16, isOutput=False)
    # rows 0:64 hold +sin, rows 64:128 hold -sin
    nsinT = nc.declare_dram_parameter("nsinT", [DH, T], bf16, isOutput=False)
    out = nc.declare_dram_parameter("out", [BT, D], bf16, isOutput=True)

    with nc.allow_low_precision(reason="bf16 fast matmul path"), \
         tile.TileContext(nc) as tc:
        with tc.tile_pool(name="persist", bufs=1) as P:
            ident = P.tile([128, 128], f32, tag="ident")
            ones_sq = P.tile([128, 128], fr, tag="ones_sq")
            ones_sqf = P.tile([128, 128], f32, tag="ones_sqf")
            ones_rf = P.tile([1, 128], f32, tag="ones_rf")
            make_identity(nc, ident[:])
            nc.gpsimd.memset(ones_sqf[:], 1.0)
            nc.vector.tensor_copy(ones_sq[:], ones_sqf[:])
            nc.gpsimd.memset(ones_rf[:], 1.0)
            # dummy exp: pull the 1.3us activation-table load into phase A
            # instead of paying it on the first real softmax exp
            dummy_e = P.tile([1, 1], f32, tag="dummy_e")
            nc.scalar.activation(dummy_e[:], ones_rf[0:1, 0:1],
                                 mybir.ActivationFunctionType.Exp)

            cos_sb = P.tile([128, T], bf16, tag="cos")
            nsin_sb = P.tile([128, T], bf16, tag="nsin")
            wo_sb = P.tile([128, HPC * D], bf16, tag="wo")
            # per-superblock tiles: fine-grained deps let attention start
            # on a superblock as soon as its projections+rope finish
            QTs = [[P.tile([128, 512], bf16, tag=f"qt{h}_{g}", name=f"qt{h}_{g}")
                    for g in range(NG)] for h in range(HPC)]
            KTs = [P.tile([128, 512], bf16, tag=f"kt{g}", name=f"kt{g}")
                   for g in range(NG)]
            Vns = [P.tile([128, 512], bf16, tag=f"vn{g}", name=f"vn{g}")
                   for g in range(NG)]
            AVTs = [[P.tile([128, 512], bf16, tag=f"avt{h}_{g}", name=f"avt{h}_{g}")
                     for g in range(NG)] for h in range(HPC)]

            # pools shared by both regions. PSUM budget is exact 16KB
            # per partition in each region:
            #   region A/2: psA 8KB + psSt2 4KB + psAv 2KB + psL 2KB
            #   region 3:   psSt3 8KB + psC 4KB + psAv 2KB + psL 2KB
            with tc.tile_pool(name="ptp", bufs=6) as PTP, \
                 tc.tile_pool(name="lap", bufs=3) as LAP, \
                 tc.tile_pool(name="rrp", bufs=2) as RRP, \
                 tc.tile_pool(name="osbp", bufs=4) as OSBP, \
                 tc.tile_pool(name="psL", bufs=1, space="PSUM") as PSL:
              # ------- phase A: QKV projections + RoPE, with the b=0
              # attention groups micro-interleaved into tq5>=4 so the
              # projection stream hides their exp-chain latency -------
              with tc.tile_pool(name="wpool", bufs=1) as WP, \
                 tc.tile_pool(name="xp", bufs=6) as XP, \
                 tc.tile_pool(name="ropetA", bufs=4) as RT2, \
                 tc.tile_pool(name="psA", bufs=1, space="PSUM") as PSA, \
                 tc.tile_pool(name="psSt2", bufs=3, space="PSUM") as PSST2:
                w_sb = WP.tile([128, ND * 4 * DH], bf16, tag="w")
                def load_weight_quarter(qi):
                    lo, hi = qi * (ND // 4), (qi + 1) * (ND // 4)
                    nc.sync.dma_start(
                        out=w_sb[:, lo * 512: hi * 512].rearrange(
                            "p (c m) -> p c m", c=hi - lo),
                        in_=wqkvT[lo * 128: hi * 128, :].rearrange(
                            "(c p) m -> p c m", p=128))

                # keep-warm matmuls: the PE would otherwise sit idle for
                # the initial weight/x DMA and pay the HAM half-clock ramp
                # when the first projections finally issue.
                for _ in range(_N_WARM):
                    wps = PSA.tile([128, 512], f32, tag="pv", name="warm")
                    nc.tensor.matmul(wps[:, 0:128], lhsT=ident[:], rhs=ident[:],
                                     start=True, stop=True)

                # only the first half-quarter of the weights before the
                # first x tile; the rest follows after the first matmuls
                def load_weight_chunks(lo, hi):
                    nc.sync.dma_start(
                        out=w_sb[:, lo * 512: hi * 512].rearrange(
                            "p (c m) -> p c m", c=hi - lo),
                        in_=wqkvT[lo * 128: hi * 128, :].rearrange(
                            "(c p) m -> p c m", p=128))
                load_weight_chunks(0, 4)

                # --- b=0 attention as a queue of micro-ops (one S+exp or
                # one mask+denominator+AV or one normalization chain each),
                # popped between projection chunks of tq5>=4. Items are
                # ordered with the AV half one slot behind its S half, so
                # the in-order PE never waits on an exp. ---
                b0q = []
                b0_state = {}

                def b0_attention_items():
                    items = []
                    for tsb in (3, 2, 1, 0):
                        g = tsb
                        n_sc = (tsb + 1) * 4
                        for h in range(HPC):
                            key = (g, h)
                            pend = []

                            def s_part(sc, g=g, h=h, tsb=tsb, key=key):
                                c0 = max(sc - tsb * 4, 0) * 128
                                if sc == 0:
                                    b0_state[key] = (
                                        PSL.tile([128, 512], f32, tag="lr",
                                                 name=f"b0av{g}_{h}"),
                                        LAP.tile([128, 512], fr, tag="pacc",
                                                 name=f"b0pacc{g}_{h}"))
                                st1 = PSST2.tile([128, 512], f32, tag="st2",
                                                 name=f"b0st{g}_{h}_{sc}")
                                pt1 = PTP.tile([128, 512], bf16, tag="pt2s",
                                               name=f"b0pt{g}_{h}_{sc}")
                                nc.tensor.matmul(
                                    st1[:, c0:512],
                                    lhsT=KTs[sc // 4][:, (sc % 4) * 128:
                                                      (sc % 4 + 1) * 128],
                                    rhs=QTs[h][g][:, c0:512],
                                    start=True, stop=True)
                                nc.scalar.activation(
                                    pt1[:, c0:512], st1[:, c0:512],
                                    mybir.ActivationFunctionType.Exp)
                                return pt1

                            def av_part(sc, pt1, g=g, h=h, tsb=tsb, key=key,
                                        n_sc=n_sc):
                                c0 = max(sc - tsb * 4, 0) * 128
                                av_ps, pacc = b0_state[key]
                                if sc >= tsb * 4:
                                    nc.gpsimd.affine_select(
                                        out=pt1[:, c0:c0 + 128],
                                        in_=pt1[:, c0:c0 + 128],
                                        compare_op=mybir.AluOpType.is_ge,
                                        fill=0.0,
                                        base=0,
                                        pattern=[[1, 128]],
                                        channel_multiplier=-1,
                                    )
                                if sc == 0:
                                    nc.gpsimd.tensor_copy(pacc[:, :], pt1[:, :])
                                else:
                                    nc.gpsimd.tensor_add(
                                        pacc[:, c0:512], pacc[:, c0:512],
                                        pt1[:, c0:512])
                                nc.tensor.matmul(
                                    av_ps[:, c0:512],
                                    lhsT=Vns[sc // 4][:, (sc % 4) * 128:
                                                      (sc % 4 + 1) * 128],
                                    rhs=pt1[:, c0:512],
                                    start=(sc == 0), stop=(sc == n_sc - 1))

                            def mk_s(sc, s_part=s_part, pend=pend):
                                def run(sc=sc, s_part=s_part, pend=pend):
                                    pt1 = s_part(sc)
                                    pend.append((sc, pt1))
                                return run

                            def mk_av(av_part=av_part, pend=pend):
                                def run(av_part=av_part, pend=pend):
                                    sc, pt1 = pend.pop(0)
                                    av_part(sc, pt1)
                                return run

                            def chain(g=g, h=h, key=key):
                                av_ps, pacc = b0_state[key]
                                nc.vector.tensor_copy(AVTs[h][g][:, :], av_ps[:])
                                l_bc = PSL.tile([128, 512], f32, tag="lr",
                                                name=f"b0l{g}_{h}")
                                nc.tensor.matmul(l_bc[:], lhsT=ones_sq[:],
                                                 rhs=pacc[:], start=True,
                                                 stop=True)
                                linv = RRP.tile([128, 512], bf16, tag="linv")
                                nc.vector.reciprocal(linv[:], l_bc[:])
                                nc.vector.tensor_mul(AVTs[h][g][:, :],
                                                     AVTs[h][g][:, :], linv[:])

                            items.append(mk_s(0))
                            for sc in range(1, n_sc):
                                items.append(mk_s(sc))
                                items.append(mk_av())
                            items.append(mk_av())
                            items.append(chain)
                    return items

                b0_points = [4 * 20]  # dci + V-pass points in tq5 4..7
                def b0_drip():
                    if not b0q:
                        return
                    pts = b0_points[0]
                    k = -(-len(b0q) // max(pts, 1))
                    for _ in range(min(k, len(b0q))):
                        b0q.pop(0)()
                    b0_points[0] = max(pts - 1, 1)

                for tq5 in range(NG):
                    pq = [PSA.tile([128, 512], f32, tag=f"pq{h}", name=f"pq{h}") for h in range(HPC)]
                    pk = PSA.tile([128, 512], f32, tag="pk")
                    # all four V accumulators share one PSUM bank (PSUM
                    # slots are bank-granular; separate accumulation chains
                    # in disjoint column ranges of a bank are fine)
                    pv = PSA.tile([128, 512], f32, tag="pv")
                    xts = []
                    for dcg in range(4):
                        # one 512KB DMA: 4 d-chunks x 512 tokens
                        xt = XP.tile([128, 4 * 512], bf16, tag="x")
                        nc.sync.dma_start(
                            out=xt[:].rearrange("p (c m) -> p c m", c=4),
                            in_=xT[dcg * 512:(dcg + 1) * 512,
                                   tq5 * 512:(tq5 + 1) * 512].rearrange(
                                       "(c p) m -> p c m", p=128))
                        # tables + wo spread through tq5=1/2 x loads: the
                        # first rope (DVE) has ~100us of slack before its
                        # consumer, so keep the DMA queue clear for x tiles
                        if tq5 == 1 and dcg == 1:
                            nc.sync.dma_start(out=cos_sb[:], in_=cosT[:, :])
                        if tq5 == 1 and dcg == 3:
                            nc.sync.dma_start(out=nsin_sb[:], in_=nsinT[:, :])
                        if tq5 == 2 and dcg in (1, 3):
                            wh = dcg // 2
                            nc.sync.dma_start(
                                out=wo_sb[:, wh * D:(wh + 1) * D],
                                in_=woT[wh * 128:(wh + 1) * 128, :])
                        xts.append(xt)
                        for dci in range(4):
                            dc = dcg * 4 + dci
                            xs = xt[:, dci * 512:(dci + 1) * 512]
                            st, sp = (dc == 0), (dc == ND - 1)
                            for h in range(HPC):
                                nc.tensor.matmul(
                                    pq[h][:],
                                    lhsT=w_sb[:, dc * 512 + h * 128: dc * 512 + (h + 1) * 128],
                                    rhs=xs, start=st, stop=sp)
                            nc.tensor.matmul(
                                pk[:], lhsT=w_sb[:, dc * 512 + 256: dc * 512 + 384],
                                rhs=xs, start=st, stop=sp)
                            b0_drip()
                        # next weight quarter AFTER this dcg's matmuls:
                        # w_sb deps are whole-tile, so emitting the
                        # prefetch first would stall them on its DMA
                        if tq5 == 0 and dcg < 3:
                            load_weight_quarter(dcg + 1)
                    # V in natural [token, dh] layout, x-chunks as lhsT.
                    # One tb's 16-chunk chain finishes before the next
                    # starts: the four accumulators share one PSUM bank and
                    # a bank allows only one open accumulation group.
                    for tb in range(4):
                        for dc in range(ND):
                            xs = xts[dc // 4][:, (dc % 4) * 512:
                                              (dc % 4 + 1) * 512]
                            nc.tensor.matmul(
                                pv[:, tb * 128:(tb + 1) * 128],
                                lhsT=xs[:, tb * 128:(tb + 1) * 128],
                                rhs=w_sb[:, dc * 512 + 384: dc * 512 + 512],
                                start=(dc == 0), stop=(dc == ND - 1))
                        b0_drip()
                    # Vn copy first so the pv bank drains promptly
                    nc.vector.tensor_copy(Vns[tq5][:, :], pv[:, :])
                    for h in range(HPC):
                        nc.vector.tensor_copy(QTs[h][tq5][:, :], pq[h][:, :])
                    nc.vector.tensor_copy(KTs[tq5][:, :], pk[:, :])
                    # RoPE is deferred one tq5 iteration: its cos/nsin
                    # tables are DMA'd during tq5=1, and the tile framework
                    # orders deps by emission, so rope(k) is emitted at the
                    # end of iteration k+1 (consumers are ~100us away).
                    # HW cannot partition-shift in a 2-input op, but a
                    # 1-input DVE copy can: multiply by nsin in place
                    # (rows 0:64 = +sin, 64:128 = -sin), then swap halves
                    # with two shifted copies to build rot*sin.
                    def emit_rope(idx):
                        tab = slice((idx * 512) % T, (idx * 512) % T + 512)
                        for tgt in [KTs[idx], QTs[0][idx], QTs[1][idx]]:
                            rtmp = RT2.tile([128, 512], bf16, tag="rtmp")
                            nc.vector.tensor_mul(rtmp[:], tgt[:, :],
                                                 nsin_sb[:, tab])
                            rtsw = RT2.tile([128, 512], bf16, tag="rtsw")
                            nc.vector.tensor_copy(rtsw[0:64, :], rtmp[64:128, :])
                            nc.vector.tensor_copy(rtsw[64:128, :], rtmp[0:64, :])
                            nc.vector.tensor_mul(tgt[:, :], tgt[:, :],
                                                 cos_sb[:, tab])
                            nc.gpsimd.tensor_add(tgt[:, :], tgt[:, :], rtsw[:])
                    if 1 <= tq5 <= 2:
                        emit_rope(tq5 - 1)
                    if tq5 == 3:
                        emit_rope(2)
                        emit_rope(3)
                    if 5 <= tq5 <= 6:
                        emit_rope(tq5 - 1)
                    if tq5 == NG - 1:
                        emit_rope(tq5 - 1)
                        emit_rope(tq5)
                    if tq5 == 3:
                        b0q.extend(b0_attention_items())
                    if tq5 == NG - 1:
                        while b0q:
                            b0q.pop(0)()
              # ------- region 3: b=1 attention (paired exp) + ALL
              # groups' output projections -------
              with tc.tile_pool(name="psC", bufs=3, space="PSUM") as PSC, \
                 tc.tile_pool(name="psSt", bufs=2, space="PSUM") as PSST:
                osb_cnt = [0]

                def wo_units(b, tsb, fine_from=99, dve_only=False):
                    # output projection for one 512-token group as 8
                    # independently emittable units (one osb half each),
                    # 512-wide d-quarters with double-buffered PSUM.
                    # fine=False DMAs each quarter separately (shorter tail).
                    g = b * NSB + tsb
                    units = []
                    for tj in range(4):
                        for dhalf in range(2):
                            def unit(tj=tj, dhalf=dhalf,
                                     fine=(tj * 2 + dhalf >= fine_from)):
                                tcx = (b * T + tsb * 512) // 128 + tj
                                osb = OSBP.tile([128, D // 2], bf16, tag="osb")
                                for dq in range(2):
                                    dcol = dhalf * 1024 + dq * 512
                                    wo_ps = PSC.tile([128, 512], f32, tag="wops")
                                    for h in range(HPC):
                                        nc.tensor.matmul(
                                            wo_ps[:],
                                            lhsT=AVTs[h][g][:, tj * 128:(tj + 1) * 128],
                                            rhs=wo_sb[:, h * D + dcol: h * D + dcol + 512],
                                            start=(h == 0), stop=(h == HPC - 1))
                                    # only DVE and Act can read PSUM; Act
                                    # also runs exp, so weight DVE 2:1 and
                                    # keep Act fully clear for units dripped
                                    # between attention pairs
                                    eng = osb_cnt[0] % 3
                                    osb_cnt[0] += 1
                                    dst = osb[:, dq * 512:(dq + 1) * 512]
                                    if eng == 2 and not dve_only:
                                        nc.scalar.copy(dst, wo_ps[:])
                                    else:
                                        nc.vector.tensor_copy(dst, wo_ps[:])
                                    if fine:
                                        nc.sync.dma_start(
                                            out=out[tcx * 128:(tcx + 1) * 128,
                                                    dcol:dcol + 512],
                                            in_=dst)
                                if not fine:
                                    nc.sync.dma_start(
                                        out=out[tcx * 128:(tcx + 1) * 128,
                                                dhalf * 1024:(dhalf + 1) * 1024],
                                        in_=osb[:])
                            units.append(unit)
                    return units

                # b=0 attention already ran inside phase A; here the b=1
                # groups run with the b=0 groups' wo blocks as stall fill,
                # two wo groups flushed after each b=1 group's heads
                # ordered by rope availability: (1,1) needs only
                # rope(4,5); (1,3) (which needs rope(7), emitted last)
                # runs third
                groups = [(1, 1), (1, 2), (1, 3), (1, 0)]
                flush_after = {(1, 1): [(0, 3), (0, 2)],
                               (1, 2): [(0, 1), (0, 0)],
                               (1, 3): [(1, 1), (1, 2)],
                               (1, 0): [(1, 3)]}
                for b, tsb in groups:
                    n_sc = (tsb + 1) * 4
                    g = b * NSB + tsb
                    # this group's fill units are all available during its
                    # own pairs (their source groups finished earlier):
                    # drip half of them into the pair loop, flush the rest
                    fill = []
                    for wg in flush_after[(b, tsb)]:
                        fill.extend(wo_units(*wg))
                    drip_budget = [len(fill) // 2]
                    def drip(fill=fill, drip_budget=drip_budget):
                        if drip_budget[0] > 0 and fill:
                            fill.pop(0)()
                            drip_budget[0] -= 1
                    for h in range(HPC):
                        av_ps = PSL.tile([128, 512], f32, tag="lr",
                                         name="av_ps")
                        pacc = LAP.tile([128, 512], fr, tag="pacc")

                        def emit_av(pt2, halves):
                            # the P-consuming half of a pair: causal-mask
                            # select, denominator accumulate, AV matmuls
                            for sc, c0, off in halves:
                                # zero the upper triangle of the diagonal
                                # 128-block post-exp on Pool (exp of
                                # unmasked scores is bounded, ~exp(1.5))
                                if sc >= tsb * 4:
                                    nc.gpsimd.affine_select(
                                        out=pt2[:, off + c0: off + c0 + 128],
                                        in_=pt2[:, off + c0: off + c0 + 128],
                                        compare_op=mybir.AluOpType.is_ge,
                                        fill=0.0,
                                        base=0,
                                        pattern=[[1, 128]],
                                        channel_multiplier=-1,
                                    )
                                # softmax denominators accumulate on Pool
                                # (saves one PE column-stream per chunk)
                                if sc == 0:
                                    nc.gpsimd.tensor_copy(pacc[:, :], pt2[:, 0:512])
                                else:
                                    nc.gpsimd.tensor_add(
                                        pacc[:, c0:512], pacc[:, c0:512],
                                        pt2[:, off + c0: off + 512])
                                nc.tensor.matmul(
                                    av_ps[:, c0:512],
                                    lhsT=Vns[b * NSB + sc // 4][:, (sc % 4) * 128:
                                                                (sc % 4 + 1) * 128],
                                    rhs=pt2[:, off + c0: off + 512],
                                    start=(sc == 0), stop=(sc == n_sc - 1))

                        # lag-2 software pipeline: pair i's AV work is
                        # emitted after pair i+1's S/exp, so on the in-order
                        # PE the AV matmuls never wait on their own exp
                        avq = []
                        for scp in range(n_sc // 2):
                            st2 = PSST.tile([128, 1024], f32, tag="st")
                            pt2 = PTP.tile([128, 1024], bf16, tag="pt")
                            halves = []
                            for k in (0, 1):
                                sc = 2 * scp + k
                                c0 = max(sc - tsb * 4, 0) * 128
                                off = k * 512
                                nc.tensor.matmul(
                                    st2[:, off + c0: off + 512],
                                    lhsT=KTs[b * NSB + sc // 4][:, (sc % 4) * 128:
                                                                (sc % 4 + 1) * 128],
                                    rhs=QTs[h][g][:, c0:512],
                                    start=True, stop=True)
                                halves.append((sc, c0, off))
                            if halves[1][1] == 0:
                                # both halves full-width: one paired exp
                                nc.scalar.activation(
                                    pt2[:, :], st2[:, :],
                                    mybir.ActivationFunctionType.Exp)
                            else:
                                for sc, c0, off in halves:
                                    nc.scalar.activation(
                                        pt2[:, off + c0: off + 512],
                                        st2[:, off + c0: off + 512],
                                        mybir.ActivationFunctionType.Exp)
                            avq.append((pt2, halves))
                            if len(avq) > 1:
                                emit_av(*avq.pop(0))
                            drip()
                        for item in avq:
                            emit_av(*item)
                        # copy first: frees av_ps for the next head's
                        # accumulation while the l chain runs
                        nc.vector.tensor_copy(AVTs[h][g][:, :], av_ps[:])
                        # all-ones lhsT: one matmul both sums pacc over s
                        # AND broadcasts l to every partition
                        l_bc = PSL.tile([128, 512], f32, tag="lr", name="l_bc")
                        nc.tensor.matmul(l_bc[:], lhsT=ones_sq[:], rhs=pacc[:],
                                         start=True, stop=True)
                        linv = RRP.tile([128, 512], bf16, tag="linv")
                        nc.vector.reciprocal(linv[:], l_bc[:])
                        nc.vector.tensor_mul(AVTs[h][g][:, :], AVTs[h][g][:, :],
                                             linv[:])
                    while fill:
                        fill.pop(0)()
                for u in wo_units(1, 0, fine_from=6):
                    u()

    if split_waits:
        _split_multi_waits(nc, mybir)
    return nc


def _host_inputs(x, wq, wk, wv, wo):
    import ml_dtypes
    bf = ml_dtypes.bfloat16
    xT = np.ascontiguousarray(x.reshape(BT, D).T.astype(bf))
    half = DH // 2
    inv = (1.0 / (ROPE_BASE ** (np.arange(half, dtype=np.float32) / half))).astype(np.float32)
    ang = np.arange(T, dtype=np.float32)[:, None] * inv[None, :]          # (T, 64)
    c = np.cos(ang).T.astype(np.float32)                                  # (64, T)
    s = np.sin(ang).T.astype(np.float32)
    cosT = np.ascontiguousarray(np.concatenate([c, c], axis=0).astype(bf))  # (128, T)
    nsinT = np.ascontiguousarray(np.concatenate([s, -s], axis=0).astype(bf))
    scale = np.float32(1.0 / np.sqrt(DH))
    in_maps = []
    for core in range(NCORES):
        kvh = core // 2
        wqkvT = np.concatenate([
            (wq[core * HPC * DH:(core + 1) * HPC * DH, :] * scale).T,
            wk[kvh * DH:(kvh + 1) * DH, :].T,
            wv[kvh * DH:(kvh + 1) * DH, :].T,
        ], axis=1)
        in_maps.append({
            "xT": xT,
            "wqkvT": np.ascontiguousarray(wqkvT.astype(bf)),
            "woT": np.ascontiguousarray(wo[:, core * HPC * DH:(core + 1) * HPC * DH].T.astype(bf)),
            "cosT": cosT,
            "nsinT": nsinT,
        })
    return in_maps


def kernel(x, wq, wk, wv, wo):
    _ensure_path()
    from concourse.bass_utils import run_bass_kernel_spmd

    x = np.asarray(x, dtype=np.float32)
    wq = np.asarray(wq, dtype=np.float32)
    wk = np.asarray(wk, dtype=np.float32)
    wv = np.asarray(wv, dtype=np.float32)
    wo = np.asarray(wo, dtype=np.float32)

    if "nc" not in _cache:
        _cache["nc"] = _build()
    nc = _cache["nc"]

    in_maps = _host_inputs(x, wq, wk, wv, wo)
    res = run_bass_kernel_spmd(nc, in_maps, list(range(NCORES)))
    acc = np.asarray(res.results[0]["out"]).astype(np.float32)
    for cidx in range(1, NCORES):
        acc = acc + np.asarray(res.results[cidx]["out"]).astype(np.float32)
    return acc.reshape(B, T, D)
